# revision 5
# baseline (speedup 1.0000x reference)
"""Causal self-attention on 8 TRN2 NeuronCores.

Sharding: core c = (batch b = c // 2, head-group g = c % 2).
Each core handles one batch and 8 of the 16 heads:
  - QKV projection for its 512 q/k/v feature slices (transposed layout)
  - causal attention for its 8 heads
  - partial output projection (its 512 rows of W_out)
Host sums the two partials per batch and adds b_out.

All TensorE matmuls run in bf16; softmax runs in f32 (exp on ScalarE,
normalization via ones-matmul column sums + VectorE reciprocal).

PE array tiling:
  - scores: the two heads of a pair are row-tiled (K=64 lhsT at partitions
    0-63 / 64-127) and run concurrently in the top/bottom array halves.
  - AV: the two heads are col-tiled (M=64 outputs at psum partitions
    0-63 / 64-127) and run concurrently in the left/right array halves.
  - softmax denominators: M=1 ones-matmuls accumulate into four distinct
    32-col groups (head x kc-parity) of one psum bank, so consecutive
    chunks' denominator matmuls also overlap.
  - normalization: denominator reciprocals for the 8 heads live on 8 SBUF
    partitions; one K=8 one-hot matmul per head pair replicates them
    across 128 psum partitions for the VectorE multiply.
"""

import numpy as np
import ml_dtypes

B, T, D, H = 4, 2048, 1024, 16
HG = 2            # head groups (tensor-parallel factor)
HL = H // HG      # 8 heads per core
HD = D // H       # 64
DG = HL * HD      # 512 features per group
SCALE = 1.0 / float(np.sqrt(HD))
NCORES = 8
TCH = T // 128    # 16 time chunks of 128
NQC = T // 512    # 4 query chunks of 512

bf16 = ml_dtypes.bfloat16

_CACHE = {}


def _split_multi_waits(nc, mybir):
    """The TPB instruction encoding has a single wait slot; this walrus build
    rejects instructions carrying more than one sync wait. Hoist extra waits
    onto standalone EventSemaphore instructions on the same engine. Tile's
    schedule is a valid serialization (waits only reference earlier-ordered
    work on other streams), so blocking the issuing stream at the same point
    cannot deadlock."""
    SKIP = ("InstTriggerDma", "InstCollectiveCompute")
    for f in nc.m.functions:
        for blk in f.blocks:
            out = []
            changed = False
            for inst in blk.instructions:
                si = getattr(inst, "sync_info", None)
                ow = list(si.on_wait) if si is not None and si.on_wait else []
                if len(ow) > 1 and type(inst).__name__ not in SKIP:
                    for i, w in enumerate(ow[:-1]):
                        out.append(mybir.InstEventSemaphore(
                            name=f"{inst.name}_hw{i}",
                            engine=inst.engine,
                            sync_info=mybir.SyncInfo(on_wait=[w], on_update=[]),
                            bass_nofuse=True,
                        ))
                    inst.sync_info = mybir.SyncInfo(
                        on_wait=[ow[-1]],
                        on_update=list(si.on_update) if si.on_update else [],
                    )
                    changed = True
                out.append(inst)
            if changed:
                blk.instructions = out


def _build_bass():
    import concourse.bass as bass
    import concourse.mybir as mybir
    import concourse.tile as tile
    from contextlib import ExitStack

    dt = mybir.dt
    f32 = dt.float32
    bf = dt.bfloat16

    nc = bass.Bass()
    xT_d = nc.declare_dram_parameter("xT", [D, T], bf, isOutput=False)
    wqk_d = nc.declare_dram_parameter("wqk", [D, 2 * DG], bf, isOutput=False)
    wv_d = nc.declare_dram_parameter("wv", [D, DG], bf, isOutput=False)
    wo_d = nc.declare_dram_parameter("wo", [DG, D], bf, isOutput=False)
    bqk_d = nc.declare_dram_parameter("bqk", [2 * DG], f32, isOutput=False)
    masks_d = nc.declare_dram_parameter("masks", [128, 4096], bf, isOutput=False)
    oh_d = nc.declare_dram_parameter("oh", [8, 4 * 128], bf, isOutput=False)
    out_d = nc.declare_dram_parameter("out", [T, D], f32, isOutput=True)

    with tile.TileContext(nc) as tc, ExitStack() as ctx:
        const = ctx.enter_context(tc.tile_pool(name="const", bufs=1))
        psum = ctx.enter_context(tc.tile_pool(name="psum", bufs=2, space="PSUM"))
        ptp = ctx.enter_context(tc.tile_pool(name="ptp", bufs=5))
        stp = ctx.enter_context(tc.tile_pool(name="stp", bufs=10))
        small = ctx.enter_context(tc.tile_pool(name="small", bufs=3))

        # ---- resident tensors --------------------------------------------
        xT_sb = const.tile([128, 8, T], bf)          # x[b].T   (feature-major)
        wqk_sb = const.tile([128, 8, 2 * DG], bf)    # W_qkv q|k columns
        wv_sb = const.tile([128, 8, DG], bf)         # W_qkv v columns
        wo_sb = const.tile([128, 4, D], bf)          # W_out rows for group
        qkT_sb = const.tile([128, 8, T], bf)         # [q^T | k^T]  (feature-major)
        vn_sb = const.tile([128, TCH, DG], bf)       # V (key-major, 8 heads x 64)
        at_sb = const.tile([128, 4, T], bf)          # A^T (normalized attn out)
        masks_sb = const.tile([128, 4096], bf)       # per-dg diagonal masks x2 heads
        bqk_sb = const.tile([128, 8], f32)
        oh_sb = const.tile([8, 4 * 128], bf)         # K=8 one-hot lhsT per pair
        onecol = const.tile([128, 1], bf)            # all-ones stationary column

        nc.vector.memset(onecol, 1.0)

        # initial loads: 3 queues; xT comes in two token-halves so the first
        # matmuls only wait for ~3MB, not the full 9.5MB working set
        for s in range(2):
            for c in range(8):
                nc.sync.dma_start(
                    out=xT_sb[:, c, s * 1024:(s + 1) * 1024],
                    in_=xT_d[c * 128:(c + 1) * 128, s * 1024:(s + 1) * 1024])
        for c in range(8):
            nc.gpsimd.dma_start(out=wv_sb[:, c, :], in_=wv_d[c * 128:(c + 1) * 128, :])
        for c in range(8):
            nc.gpsimd.dma_start(out=wqk_sb[:, c, :], in_=wqk_d[c * 128:(c + 1) * 128, :])
        nc.scalar.dma_start(out=bqk_sb, in_=bqk_d[:].rearrange("(c p) -> p c", p=128))
        nc.scalar.dma_start(out=oh_sb, in_=oh_d[:, :])
        nc.scalar.dma_start(out=masks_sb, in_=masks_d[:, :])
        for c in range(4):
            nc.scalar.dma_start(out=wo_sb[:, c, :], in_=wo_d[c * 128:(c + 1) * 128, :])

        def qkv_v_chunk(tn):
            pv = psum.tile([128, 512], f32, tag="mm512", name=f"pv{tn}")
            for k in range(8):
                nc.tensor.matmul(
                    pv,
                    lhsT=xT_sb[:, k, tn * 128:(tn + 1) * 128],
                    rhs=wv_sb[:, k, :],
                    start=(k == 0), stop=(k == 7),
                )
            nc.vector.tensor_copy(out=vn_sb[:, tn, :], in_=pv)

        def qkv_qk_chunk(m, n):
            """Produce qkT features m*128..m*128+128 for token slice n."""
            pq = psum.tile([128, 512], f32, tag="mm512", name=f"pq{m}_{n}")
            for k in range(8):
                nc.tensor.matmul(
                    pq,
                    lhsT=wqk_sb[:, k, m * 128:(m + 1) * 128],
                    rhs=xT_sb[:, k, n * 512:(n + 1) * 512],
                    start=(k == 0), stop=(k == 7),
                )
            # psum -> SBUF bf16 with per-partition bias add, on VectorE so the
            # ScalarE stays dedicated to softmax exps
            nc.vector.tensor_scalar_add(
                out=qkT_sb[:, m, n * 512:(n + 1) * 512],
                in0=pq, scalar1=bqk_sb[:, m:m + 1],
            )

        # ---- attention (interleaved with QKV production) -----------------
        stages = {}
        ge8 = {}
        go8 = {}

        def attn(qc, p, fill=None):
            """Scores + AV + denominators for head pair p of query chunk qc.

            Each score group is one kc for both heads ([128, 1024] psum, two
            concurrent row-tiled K=64 matmuls); ScalarE exps it into pt.
            AV (col-tiled pair) and denominator matmuls for groups g-2, g-1
            are emitted after group g's score matmuls so the PE stream always
            has work while exp runs."""
            nkc = 4 * qc + 4
            h0, h1 = 2 * p, 2 * p + 1
            if p == 0:
                ge8[qc] = stp.tile([8, 512], f32, tag="ge8", bufs=2,
                                   name=f"ge8_{qc}")
                go8[qc] = stp.tile([8, 512], f32, tag="go8", bufs=2,
                                   name=f"go8_{qc}")
            qsl0 = qkT_sb[0:64, p, qc * 512:(qc + 1) * 512]
            qsl1 = qkT_sb[64:128, p, qc * 512:(qc + 1) * 512]
            pts = []
            pav = psum.tile([128, 512], f32, tag="av", name=f"pav{qc}_{p}")
            pones = psum.tile([128, 512], f32, tag="av", name=f"pones{qc}_{p}")

            def av_group(kc):
                nc.tensor.matmul(
                    pav[0:64, :],
                    lhsT=vn_sb[:, kc, h0 * HD:(h0 + 1) * HD],
                    rhs=pts[kc][:, 0:512],
                    start=(kc == 0), stop=(kc == nkc - 1),
                )
                nc.tensor.matmul(
                    pav[64:128, :],
                    lhsT=vn_sb[:, kc, h1 * HD:(h1 + 1) * HD],
                    rhs=pts[kc][:, 512:1024],
                    start=(kc == 0), stop=(kc == nkc - 1),
                )

            def ones_group(kc):
                # head x kc-parity -> 4 distinct col groups; the 4 matmuls of
                # two consecutive kc run concurrently as col tiles
                j0, j1 = kc % 2, 2 + kc % 2
                nc.tensor.matmul(
                    pones[32 * j0:32 * j0 + 1, :],
                    lhsT=onecol, rhs=pts[kc][:, 0:512],
                    start=(kc < 2), stop=(kc >= nkc - 2),
                    tile_position=(0, 32 * j0),
                )
                nc.tensor.matmul(
                    pones[32 * j1:32 * j1 + 1, :],
                    lhsT=onecol, rhs=pts[kc][:, 512:1024],
                    start=(kc < 2), stop=(kc >= nkc - 2),
                    tile_position=(0, 32 * j1),
                )

            def batch(kc):
                av_group(kc)
                av_group(kc + 1)
                ones_group(kc)
                ones_group(kc + 1)
                if fill:
                    fill.pop(0)()

            for kc in range(nkc):
                dg = kc - (nkc - 4)  # 0..3 on the masked diagonal band
                ps = psum.tile([128, 1024], f32, tag="s", name=f"ps{qc}_{p}_{kc}")
                nc.tensor.matmul(
                    ps[:, 0:512],
                    lhsT=qkT_sb[0:64, 4 + p, kc * 128:(kc + 1) * 128],
                    rhs=qsl0, start=True, stop=True,
                )
                nc.tensor.matmul(
                    ps[:, 512:1024],
                    lhsT=qkT_sb[64:128, 4 + p, kc * 128:(kc + 1) * 128],
                    rhs=qsl1, start=True, stop=True,
                )
                if kc > 0 and kc % 2 == 0:
                    batch(kc - 2)
                pt = ptp.tile([128, 1024], bf, tag="pt", name=f"pt{qc}_{p}_{kc}")
                ptv = pt.rearrange("p (h c) -> p h c", c=512)
                psv = ps.rearrange("p (h c) -> p h c", c=512)
                if dg >= 1:
                    # diagonal tiles: exp only the causally-live columns
                    lo = 128 * dg
                    nc.vector.memset(ptv[:, :, 0:lo], 0.0)
                    nc.scalar.activation(
                        out=ptv[:, :, lo:512], in_=psv[:, :, lo:512],
                        func=mybir.ActivationFunctionType.Exp, scale=SCALE,
                    )
                else:
                    nc.scalar.activation(
                        out=pt, in_=ps,
                        func=mybir.ActivationFunctionType.Exp, scale=SCALE,
                    )
                if dg >= 0:
                    # only 128 columns per head straddle the diagonal; the
                    # rest are fully live (or already memset to zero)
                    lo = 128 * dg
                    mv = masks_sb[:, dg * 1024:(dg + 1) * 1024].rearrange(
                        "p (h c) -> p h c", c=512)
                    nc.vector.tensor_mul(
                        out=ptv[:, :, lo:lo + 128], in0=ptv[:, :, lo:lo + 128],
                        in1=mv[:, :, lo:lo + 128],
                    )
                pts.append(pt)
            batch(nkc - 2)

            stage = stp.tile([128, 512], bf, tag="stage", bufs=6,
                             name=f"st{qc}_{p}")
            nc.vector.tensor_copy(out=stage, in_=pav)
            stages[(qc, p)] = stage
            # denominator partials (4 col-group rows) -> SBUF, then gather the
            # strided rows into head-major order for the VectorE add
            pone_sb = stp.tile([97, 512], f32, tag="pone", bufs=2,
                               name=f"pone{qc}_{p}")
            nc.vector.tensor_copy(out=pone_sb, in_=pones[0:97, :])
            nc.gpsimd.dma_start(out=ge8[qc][2 * p:2 * p + 1, :], in_=pone_sb[0:1, :])
            nc.gpsimd.dma_start(out=go8[qc][2 * p:2 * p + 1, :], in_=pone_sb[32:33, :])
            nc.gpsimd.dma_start(out=ge8[qc][2 * p + 1:2 * p + 2, :], in_=pone_sb[64:65, :])
            nc.gpsimd.dma_start(out=go8[qc][2 * p + 1:2 * p + 2, :], in_=pone_sb[96:97, :])

        def divisions(qc):
            coll = stp.tile([8, 512], f32, tag="coll", bufs=2, name=f"coll{qc}")
            nc.vector.tensor_add(out=coll, in0=ge8[qc], in1=go8[qc])
            rcoll = stp.tile([8, 512], f32, tag="rcoll", bufs=2)
            nc.vector.reciprocal(rcoll, coll)
            rcollb = stp.tile([8, 512], bf, tag="rcollb", bufs=2)
            nc.vector.tensor_copy(out=rcollb, in_=rcoll)
            for p in range(4):
                # replicate rows (2p, 2p+1) of rcollb across 64 psum
                # partitions each via a K=8 one-hot stationary matmul
                prb = psum.tile([128, 512], f32, tag="mm512", name=f"prb{qc}_{p}")
                nc.tensor.matmul(
                    prb, lhsT=oh_sb[:, p * 128:(p + 1) * 128], rhs=rcollb,
                    start=True, stop=True,
                )
                nc.vector.tensor_mul(
                    out=at_sb[:, p, qc * 512:(qc + 1) * 512],
                    in0=stages[(qc, p)], in1=prb,
                )

        def outproj_unit(qj, dn):
            def emit():
                po = psum.tile([128, 512], f32, tag="mm512",
                               name=f"po{qj}_{dn}")
                for kc in range(4):
                    nc.tensor.matmul(
                        po,
                        lhsT=at_sb[:, kc, qj * 128:(qj + 1) * 128],
                        rhs=wo_sb[:, kc, dn * 512:(dn + 1) * 512],
                        start=(kc == 0), stop=(kc == 3),
                    )
                ost = small.tile([128, 512], f32, tag="ost")
                nc.vector.tensor_copy(out=ost, in_=po)
                nc.sync.dma_start(
                    out=out_d[qj * 128:(qj + 1) * 128,
                              dn * 512:(dn + 1) * 512],
                    in_=ost,
                )
            return emit

        def outproj_units(qc):
            return [outproj_unit(qj, dn)
                    for qj in range(4 * qc, 4 * qc + 4) for dn in range(2)]

        # qkT production is sliced by token chunk n and emitted just ahead of
        # the attention block that consumes it, so ScalarE exp work starts
        # early and paces the whole kernel; spare PE slots inside attention
        # blocks are filled from a unit queue (V chunks, outproj, leftovers).
        for tn in range(4):
            qkv_v_chunk(tn)
        fq = [(lambda t: (lambda: qkv_v_chunk(t)))(tn) for tn in range(4, 8)]
        for p in range(4):
            qkv_qk_chunk(p, 0)
            qkv_qk_chunk(4 + p, 0)
            attn(0, p, fq)
        for u in fq:
            u()
        fq = [(lambda t: (lambda: qkv_v_chunk(t)))(tn) for tn in range(8, 12)]
        for p in range(4):
            qkv_qk_chunk(p, 1)
            qkv_qk_chunk(4 + p, 1)
            attn(1, p, fq)
            if p == 1:
                divisions(0)
        for u in fq:
            u()
        fq = [(lambda t: (lambda: qkv_v_chunk(t)))(tn) for tn in range(12, 16)]
        fq += outproj_units(0)
        for p in range(4):
            qkv_qk_chunk(p, 2)
            qkv_qk_chunk(4 + p, 2)
            attn(2, p, fq)
            if p == 1:
                divisions(1)
        for u in fq[:4]:
            u()
        fq = fq[4:] + outproj_units(1)
        for p in range(4):
            qkv_qk_chunk(p, 3)
            qkv_qk_chunk(4 + p, 3)
            attn(3, p, fq)
            if p == 1:
                divisions(2)
            if p == 2:
                fq += outproj_units(2)
        divisions(3)
        for u in fq:
            u()
        for u in outproj_units(3):
            u()

    _split_multi_waits(nc, mybir)
    return nc


def _make_masks():
    kl = np.arange(128)[:, None]
    ql = np.arange(512)[None, :]
    t = [(ql >= kl + 128 * i).astype(np.float32) for i in range(4)]
    # block dg holds the mask for diagonal offset 128*dg, duplicated for the
    # two heads packed side by side in each [128, 1024] score group
    return np.concatenate([np.concatenate([m, m], axis=1) for m in t],
                          axis=1).astype(bf16)  # [128, 4096]


def _make_in_maps(x, W_qkv, b_qkv, W_out):
    masks = _make_masks()
    # oh[k, 128p + c] = (k == 2p + (c >= 64)): K=8 one-hot stationary that
    # replicates head-pair p's two reciprocal rows across 128 psum partitions
    oh = np.zeros((8, 4, 128), np.float32)
    for p in range(4):
        oh[2 * p, p, 0:64] = 1.0
        oh[2 * p + 1, p, 64:128] = 1.0
    oh = oh.reshape(8, 4 * 128).astype(bf16)
    in_maps = []
    for c in range(NCORES):
        b, g = divmod(c, 2)
        xT = np.ascontiguousarray(x[b].T).astype(bf16)
        wq = W_qkv[:, g * DG:(g + 1) * DG]
        wk = W_qkv[:, D + g * DG:D + (g + 1) * DG]
        wv = W_qkv[:, 2 * D + g * DG:2 * D + (g + 1) * DG]
        wqk = np.concatenate([wq, wk], axis=1).astype(bf16)
        bq = b_qkv[g * DG:(g + 1) * DG]
        bk = b_qkv[D + g * DG:D + (g + 1) * DG]
        bqk = np.concatenate([bq, bk]).astype(np.float32)
        wo = W_out[g * DG:(g + 1) * DG, :].astype(bf16)
        in_maps.append({
            "xT": xT,
            "wqk": wqk,
            "wv": np.ascontiguousarray(wv).astype(bf16),
            "wo": np.ascontiguousarray(wo),
            "bqk": bqk,
            "masks": masks,
            "oh": oh,
        })
    return in_maps


def _np_fallback(x, W_qkv, b_qkv, W_out, b_out):
    out = np.empty((B, T, D), np.float32)
    qkv = x.reshape(B * T, D) @ W_qkv + b_qkv
    q, k, v = np.split(qkv.reshape(B, T, 3 * D), 3, axis=-1)

    def heads(z):
        return z.reshape(B, T, H, HD).transpose(0, 2, 1, 3)

    q, k, v = heads(q), heads(k), heads(v)
    causal = np.tril(np.ones((T, T), dtype=bool))
    acc = np.empty((B, H, T, HD), np.float32)
    for bi in range(B):
        for h in range(H):
            s = (q[bi, h] @ k[bi, h].T) * np.float32(SCALE)
            s = np.where(causal, s, -np.inf)
            s -= s.max(axis=-1, keepdims=True)
            p = np.exp(s)
            p /= p.sum(axis=-1, keepdims=True)
            acc[bi, h] = p @ v[bi, h]
    a = acc.transpose(0, 2, 1, 3).reshape(B, T, D)
    for bi in range(B):
        out[bi] = a[bi] @ W_out + b_out
    return out


def run(x, W_qkv, b_qkv, W_out, b_out, trace=False, trace_kwargs=None):
    from concourse import bass_utils

    x = np.asarray(x, np.float32)
    W_qkv = np.asarray(W_qkv, np.float32)
    b_qkv = np.asarray(b_qkv, np.float32)
    W_out = np.asarray(W_out, np.float32)
    b_out = np.asarray(b_out, np.float32)

    # the on-device kernel folds b_qkv's q/k slices in; its v slice is
    # assumed zero (true for this problem family). Fall back if not.
    if np.any(b_qkv[2 * D:]):
        return _np_fallback(x, W_qkv, b_qkv, W_out, b_out), None

    if "nc" not in _CACHE:
        _CACHE["nc"] = _build_bass()
    nc = _CACHE["nc"]

    in_maps = _make_in_maps(x, W_qkv, b_qkv, W_out)
    kw = dict(trace=trace)
    if trace_kwargs:
        kw.update(trace_kwargs)
    res = bass_utils.run_bass_kernel_spmd(nc, in_maps, list(range(NCORES)), **kw)

    out = np.empty((B, T, D), np.float32)
    for b in range(B):
        out[b] = (np.asarray(res.results[2 * b]["out"], np.float32)
                  + np.asarray(res.results[2 * b + 1]["out"], np.float32)
                  + b_out)
    return out, res


def kernel(x, W_qkv, b_qkv, W_out, b_out):
    out, _ = run(x, W_qkv, b_qkv, W_out, b_out, trace=False)
    return out


# revision 7
# speedup vs baseline: 1.0953x; 1.0953x over previous
"""Causal self-attention on 8 TRN2 NeuronCores.

Sharding: core c = (batch b = c // 2, head-group g = c % 2).
Each core handles one batch and 8 of the 16 heads:
  - QKV projection for its 512 q/k/v feature slices (transposed layout)
  - causal attention for its 8 heads
  - partial output projection (its 512 rows of W_out)
Host sums the two partials per batch and adds b_out.

All TensorE matmuls run in bf16; softmax runs in f32 (exp on ScalarE,
normalization via ones-matmul column sums + VectorE reciprocal).

PE array tiling:
  - scores: the two heads of a pair are row-tiled (K=64 lhsT at partitions
    0-63 / 64-127) and run concurrently in the top/bottom array halves.
  - AV: the two heads are col-tiled (M=64 outputs at psum partitions
    0-63 / 64-127) and run concurrently in the left/right array halves.
  - softmax denominators: M=1 ones-matmuls accumulate into four distinct
    32-col groups (head x kc-parity) of one psum bank, so consecutive
    chunks' denominator matmuls also overlap.
  - normalization: denominator reciprocals for the 8 heads live on 8 SBUF
    partitions; one K=8 one-hot matmul per head pair replicates them
    across 128 psum partitions for the VectorE multiply.
"""

import numpy as np
import ml_dtypes

B, T, D, H = 4, 2048, 1024, 16
HG = 2            # head groups (tensor-parallel factor)
HL = H // HG      # 8 heads per core
HD = D // H       # 64
DG = HL * HD      # 512 features per group
SCALE = 1.0 / float(np.sqrt(HD))
NCORES = 8
TCH = T // 128    # 16 time chunks of 128
NQC = T // 512    # 4 query chunks of 512

bf16 = ml_dtypes.bfloat16

_CACHE = {}


def _split_multi_waits(nc, mybir):
    """The TPB instruction encoding has a single wait slot; this walrus build
    rejects instructions carrying more than one sync wait. Hoist extra waits
    onto standalone EventSemaphore instructions on the same engine. Tile's
    schedule is a valid serialization (waits only reference earlier-ordered
    work on other streams), so blocking the issuing stream at the same point
    cannot deadlock."""
    SKIP = ("InstTriggerDma", "InstCollectiveCompute")
    for f in nc.m.functions:
        for blk in f.blocks:
            out = []
            changed = False
            for inst in blk.instructions:
                si = getattr(inst, "sync_info", None)
                ow = list(si.on_wait) if si is not None and si.on_wait else []
                if len(ow) > 1 and type(inst).__name__ not in SKIP:
                    for i, w in enumerate(ow[:-1]):
                        out.append(mybir.InstEventSemaphore(
                            name=f"{inst.name}_hw{i}",
                            engine=inst.engine,
                            sync_info=mybir.SyncInfo(on_wait=[w], on_update=[]),
                            bass_nofuse=True,
                        ))
                    inst.sync_info = mybir.SyncInfo(
                        on_wait=[ow[-1]],
                        on_update=list(si.on_update) if si.on_update else [],
                    )
                    changed = True
                out.append(inst)
            if changed:
                blk.instructions = out


def _build_bass():
    import concourse.bass as bass
    import concourse.mybir as mybir
    import concourse.tile as tile
    from contextlib import ExitStack

    dt = mybir.dt
    f32 = dt.float32
    bf = dt.bfloat16

    nc = bass.Bass()
    xT_d = nc.declare_dram_parameter("xT", [D, T], bf, isOutput=False)
    wqk_d = nc.declare_dram_parameter("wqk", [D, 2 * DG], bf, isOutput=False)
    wv_d = nc.declare_dram_parameter("wv", [D, DG], bf, isOutput=False)
    wo_d = nc.declare_dram_parameter("wo", [DG, D], bf, isOutput=False)
    bqk_d = nc.declare_dram_parameter("bqk", [2 * DG], f32, isOutput=False)
    masks_d = nc.declare_dram_parameter("masks", [128, 4096], bf, isOutput=False)
    oh_d = nc.declare_dram_parameter("oh", [8, 4 * 128], bf, isOutput=False)
    out_d = nc.declare_dram_parameter("out", [T, D], f32, isOutput=True)

    with tile.TileContext(nc) as tc, ExitStack() as ctx:
        const = ctx.enter_context(tc.tile_pool(name="const", bufs=1))
        psum = ctx.enter_context(tc.tile_pool(name="psum", bufs=2, space="PSUM"))
        ptp = ctx.enter_context(tc.tile_pool(name="ptp", bufs=5))
        stp = ctx.enter_context(tc.tile_pool(name="stp", bufs=10))
        small = ctx.enter_context(tc.tile_pool(name="small", bufs=3))

        # ---- resident tensors --------------------------------------------
        xT_sb = const.tile([128, 8, T], bf)          # x[b].T   (feature-major)
        wqk_sb = const.tile([128, 8, 2 * DG], bf)    # W_qkv q|k columns
        wv_sb = const.tile([128, 8, DG], bf)         # W_qkv v columns
        wo_sb = const.tile([128, 4, D], bf)          # W_out rows for group
        qkT_sb = const.tile([128, 8, T], bf)         # [q^T | k^T]  (feature-major)
        vn_sb = const.tile([128, TCH, DG], bf)       # V (key-major, 8 heads x 64)
        at_sb = const.tile([128, 4, T], bf)          # A^T (normalized attn out)
        masks_sb = const.tile([128, 4096], bf)       # per-dg diagonal masks x2 heads
        bqk_sb = const.tile([128, 8], f32)
        oh_sb = const.tile([8, 4 * 128], bf)         # K=8 one-hot lhsT per pair
        onecol = const.tile([128, 1], bf)            # all-ones stationary column

        nc.vector.memset(onecol, 1.0)

        # initial loads: 3 queues; xT comes in two token-halves so the first
        # matmuls only wait for ~3MB, not the full 9.5MB working set
        for s in range(2):
            for c in range(8):
                nc.sync.dma_start(
                    out=xT_sb[:, c, s * 1024:(s + 1) * 1024],
                    in_=xT_d[c * 128:(c + 1) * 128, s * 1024:(s + 1) * 1024])
        for c in range(8):
            nc.gpsimd.dma_start(out=wv_sb[:, c, :], in_=wv_d[c * 128:(c + 1) * 128, :])
        for c in range(8):
            nc.gpsimd.dma_start(out=wqk_sb[:, c, :], in_=wqk_d[c * 128:(c + 1) * 128, :])
        nc.scalar.dma_start(out=bqk_sb, in_=bqk_d[:].rearrange("(c p) -> p c", p=128))
        nc.scalar.dma_start(out=oh_sb, in_=oh_d[:, :])
        nc.scalar.dma_start(out=masks_sb, in_=masks_d[:, :])
        for c in range(4):
            nc.scalar.dma_start(out=wo_sb[:, c, :], in_=wo_d[c * 128:(c + 1) * 128, :])

        def qkv_v_chunk(tn):
            pv = psum.tile([128, 512], f32, tag="mm512", name=f"pv{tn}")
            for k in range(8):
                nc.tensor.matmul(
                    pv,
                    lhsT=xT_sb[:, k, tn * 128:(tn + 1) * 128],
                    rhs=wv_sb[:, k, :],
                    start=(k == 0), stop=(k == 7),
                )
            nc.vector.tensor_copy(out=vn_sb[:, tn, :], in_=pv)

        def qkv_qk_chunk(m, n):
            """Produce qkT features m*128..m*128+128 for token slice n."""
            pq = psum.tile([128, 512], f32, tag="mm512", name=f"pq{m}_{n}")
            for k in range(8):
                nc.tensor.matmul(
                    pq,
                    lhsT=wqk_sb[:, k, m * 128:(m + 1) * 128],
                    rhs=xT_sb[:, k, n * 512:(n + 1) * 512],
                    start=(k == 0), stop=(k == 7),
                )
            # psum -> SBUF bf16 with per-partition bias add, on VectorE so the
            # ScalarE stays dedicated to softmax exps
            nc.vector.tensor_scalar_add(
                out=qkT_sb[:, m, n * 512:(n + 1) * 512],
                in0=pq, scalar1=bqk_sb[:, m:m + 1],
            )

        # ---- attention (interleaved with QKV production) -----------------
        stages = {}
        ge8 = {}
        go8 = {}

        def attn(qc, p, fill=None):
            """Scores + AV + denominators for head pair p of query chunk qc.

            Each score group is one kc for both heads ([128, 1024] psum, two
            concurrent row-tiled K=64 matmuls); ScalarE exps it into pt.
            AV (col-tiled pair) and denominator matmuls for groups g-2, g-1
            are emitted after group g's score matmuls so the PE stream always
            has work while exp runs."""
            nkc = 4 * qc + 4
            h0, h1 = 2 * p, 2 * p + 1
            if p == 0:
                ge8[qc] = stp.tile([8, 512], f32, tag="ge8", bufs=2,
                                   name=f"ge8_{qc}")
                go8[qc] = stp.tile([8, 512], f32, tag="go8", bufs=2,
                                   name=f"go8_{qc}")
            qsl0 = qkT_sb[0:64, p, qc * 512:(qc + 1) * 512]
            qsl1 = qkT_sb[64:128, p, qc * 512:(qc + 1) * 512]
            pts = []
            ptsums = []
            pav = psum.tile([128, 512], f32, tag="av", name=f"pav{qc}_{p}")
            pones = psum.tile([128, 512], f32, tag="av", name=f"pones{qc}_{p}")

            def av_group(kc):
                dg = kc - (nkc - 4)
                lo = 128 * dg if dg >= 1 else 0
                nc.tensor.matmul(
                    pav[0:64, lo:512],
                    lhsT=vn_sb[:, kc, h0 * HD:(h0 + 1) * HD],
                    rhs=pts[kc][:, lo:512],
                    start=(kc == 0), stop=(kc == nkc - 1),
                )
                nc.tensor.matmul(
                    pav[64:128, lo:512],
                    lhsT=vn_sb[:, kc, h1 * HD:(h1 + 1) * HD],
                    rhs=pts[kc][:, 512 + lo:1024],
                    start=(kc == 0), stop=(kc == nkc - 1),
                )

            def ones_pair(j):
                # denominator matmuls over the GpSimd-precomputed pt pair sum;
                # head x pair-parity -> 4 distinct col groups (psum rows
                # 0/32/64/96) so consecutive pairs' matmuls overlap
                np_ = nkc // 2
                j0, j1 = j % 2, 2 + j % 2
                nc.tensor.matmul(
                    pones[32 * j0:32 * j0 + 1, :],
                    lhsT=onecol, rhs=ptsums[j][:, 0:512],
                    start=(j < 2), stop=(j >= np_ - 2),
                    tile_position=(0, 32 * j0),
                )
                nc.tensor.matmul(
                    pones[32 * j1:32 * j1 + 1, :],
                    lhsT=onecol, rhs=ptsums[j][:, 512:1024],
                    start=(j < 2), stop=(j >= np_ - 2),
                    tile_position=(0, 32 * j1),
                )

            def batch(kc):
                ptsum = ptp.tile([128, 1024], bf, tag="ptsum", bufs=3,
                                 name=f"pts{qc}_{p}_{kc}")
                nc.gpsimd.tensor_add(out=ptsum, in0=pts[kc], in1=pts[kc + 1])
                ptsums.append(ptsum)
                av_group(kc)
                av_group(kc + 1)
                if kc >= 2:
                    # trail by one batch so the PE never waits on GpSimd
                    ones_pair(kc // 2 - 1)
                if fill:
                    fill.pop(0)()

            for kc in range(nkc):
                dg = kc - (nkc - 4)  # 0..3 on the masked diagonal band
                slo = 128 * dg if dg >= 1 else 0
                ps = psum.tile([128, 1024], f32, tag="s", name=f"ps{qc}_{p}_{kc}")
                nc.tensor.matmul(
                    ps[:, slo:512],
                    lhsT=qkT_sb[0:64, 4 + p, kc * 128:(kc + 1) * 128],
                    rhs=qsl0[:, slo:512], start=True, stop=True,
                )
                nc.tensor.matmul(
                    ps[:, 512 + slo:1024],
                    lhsT=qkT_sb[64:128, 4 + p, kc * 128:(kc + 1) * 128],
                    rhs=qsl1[:, slo:512], start=True, stop=True,
                )
                if kc > 0 and kc % 2 == 0:
                    batch(kc - 2)
                pt = ptp.tile([128, 1024], bf, tag="pt", name=f"pt{qc}_{p}_{kc}")
                ptv = pt.rearrange("p (h c) -> p h c", c=512)
                psv = ps.rearrange("p (h c) -> p h c", c=512)
                if dg >= 1:
                    # diagonal tiles: exp only the causally-live columns
                    lo = 128 * dg
                    nc.vector.memset(ptv[:, :, 0:lo], 0.0)
                    nc.scalar.activation(
                        out=ptv[:, :, lo:512], in_=psv[:, :, lo:512],
                        func=mybir.ActivationFunctionType.Exp, scale=SCALE,
                    )
                else:
                    nc.scalar.activation(
                        out=pt, in_=ps,
                        func=mybir.ActivationFunctionType.Exp, scale=SCALE,
                    )
                if dg >= 0:
                    # only 128 columns per head straddle the diagonal; the
                    # rest are fully live (or already memset to zero)
                    lo = 128 * dg
                    mv = masks_sb[:, dg * 1024:(dg + 1) * 1024].rearrange(
                        "p (h c) -> p h c", c=512)
                    nc.vector.tensor_mul(
                        out=ptv[:, :, lo:lo + 128], in0=ptv[:, :, lo:lo + 128],
                        in1=mv[:, :, lo:lo + 128],
                    )
                pts.append(pt)
            batch(nkc - 2)
            ones_pair(nkc // 2 - 1)

            stage = stp.tile([128, 512], bf, tag="stage", bufs=6,
                             name=f"st{qc}_{p}")
            nc.vector.tensor_copy(out=stage, in_=pav)
            stages[(qc, p)] = stage
            # denominator partials (4 col-group rows) -> SBUF, then gather the
            # strided rows into head-major order for the VectorE add
            pone_sb = stp.tile([97, 512], f32, tag="pone", bufs=2,
                               name=f"pone{qc}_{p}")
            nc.vector.tensor_copy(out=pone_sb, in_=pones[0:97, :])
            nc.gpsimd.dma_start(out=ge8[qc][2 * p:2 * p + 1, :], in_=pone_sb[0:1, :])
            nc.gpsimd.dma_start(out=go8[qc][2 * p:2 * p + 1, :], in_=pone_sb[32:33, :])
            nc.gpsimd.dma_start(out=ge8[qc][2 * p + 1:2 * p + 2, :], in_=pone_sb[64:65, :])
            nc.gpsimd.dma_start(out=go8[qc][2 * p + 1:2 * p + 2, :], in_=pone_sb[96:97, :])

        def divisions(qc):
            coll = stp.tile([8, 512], f32, tag="coll", bufs=2, name=f"coll{qc}")
            nc.vector.tensor_add(out=coll, in0=ge8[qc], in1=go8[qc])
            rcoll = stp.tile([8, 512], f32, tag="rcoll", bufs=2)
            nc.vector.reciprocal(rcoll, coll)
            rcollb = stp.tile([8, 512], bf, tag="rcollb", bufs=2)
            nc.vector.tensor_copy(out=rcollb, in_=rcoll)
            for p in range(4):
                # replicate rows (2p, 2p+1) of rcollb across 64 psum
                # partitions each via a K=8 one-hot stationary matmul
                prb = psum.tile([128, 512], f32, tag="mm512", name=f"prb{qc}_{p}")
                nc.tensor.matmul(
                    prb, lhsT=oh_sb[:, p * 128:(p + 1) * 128], rhs=rcollb,
                    start=True, stop=True,
                )
                nc.vector.tensor_mul(
                    out=at_sb[:, p, qc * 512:(qc + 1) * 512],
                    in0=stages[(qc, p)], in1=prb,
                )

        def outproj_unit(qj, dn):
            def emit():
                po = psum.tile([128, 512], f32, tag="mm512",
                               name=f"po{qj}_{dn}")
                for kc in range(4):
                    nc.tensor.matmul(
                        po,
                        lhsT=at_sb[:, kc, qj * 128:(qj + 1) * 128],
                        rhs=wo_sb[:, kc, dn * 512:(dn + 1) * 512],
                        start=(kc == 0), stop=(kc == 3),
                    )
                ost = small.tile([128, 512], f32, tag="ost")
                nc.vector.tensor_copy(out=ost, in_=po)
                nc.sync.dma_start(
                    out=out_d[qj * 128:(qj + 1) * 128,
                              dn * 512:(dn + 1) * 512],
                    in_=ost,
                )
            return emit

        def outproj_units(qc):
            return [outproj_unit(qj, dn)
                    for qj in range(4 * qc, 4 * qc + 4) for dn in range(2)]

        # qkT production is sliced by token chunk n and emitted just ahead of
        # the attention block that consumes it, so ScalarE exp work starts
        # early and paces the whole kernel; spare PE slots inside attention
        # blocks are filled from a unit queue (V chunks, outproj, leftovers).
        for tn in range(4):
            qkv_v_chunk(tn)
        fq = [(lambda t: (lambda: qkv_v_chunk(t)))(tn) for tn in range(4, 8)]
        for p in range(4):
            qkv_qk_chunk(p, 0)
            qkv_qk_chunk(4 + p, 0)
            attn(0, p, fq)
        for u in fq:
            u()
        fq = [(lambda t: (lambda: qkv_v_chunk(t)))(tn) for tn in range(8, 12)]
        for p in range(4):
            qkv_qk_chunk(p, 1)
            qkv_qk_chunk(4 + p, 1)
            attn(1, p, fq)
            if p == 1:
                divisions(0)
        for u in fq:
            u()
        fq = [(lambda t: (lambda: qkv_v_chunk(t)))(tn) for tn in range(12, 16)]
        fq += outproj_units(0)
        for p in range(4):
            qkv_qk_chunk(p, 2)
            qkv_qk_chunk(4 + p, 2)
            attn(2, p, fq)
            if p == 1:
                divisions(1)
        for u in fq[:4]:
            u()
        fq = fq[4:] + outproj_units(1)
        for p in range(4):
            qkv_qk_chunk(p, 3)
            qkv_qk_chunk(4 + p, 3)
            attn(3, p, fq)
            if p == 1:
                divisions(2)
            if p == 2:
                fq += outproj_units(2)
        divisions(3)
        for u in fq:
            u()
        for u in outproj_units(3):
            u()

    _split_multi_waits(nc, mybir)
    return nc


def _make_masks():
    kl = np.arange(128)[:, None]
    ql = np.arange(512)[None, :]
    t = [(ql >= kl + 128 * i).astype(np.float32) for i in range(4)]
    # block dg holds the mask for diagonal offset 128*dg, duplicated for the
    # two heads packed side by side in each [128, 1024] score group
    return np.concatenate([np.concatenate([m, m], axis=1) for m in t],
                          axis=1).astype(bf16)  # [128, 4096]


def _make_in_maps(x, W_qkv, b_qkv, W_out):
    masks = _make_masks()
    # oh[k, 128p + c] = (k == 2p + (c >= 64)): K=8 one-hot stationary that
    # replicates head-pair p's two reciprocal rows across 128 psum partitions
    oh = np.zeros((8, 4, 128), np.float32)
    for p in range(4):
        oh[2 * p, p, 0:64] = 1.0
        oh[2 * p + 1, p, 64:128] = 1.0
    oh = oh.reshape(8, 4 * 128).astype(bf16)
    in_maps = []
    for c in range(NCORES):
        b, g = divmod(c, 2)
        xT = np.ascontiguousarray(x[b].T).astype(bf16)
        wq = W_qkv[:, g * DG:(g + 1) * DG]
        wk = W_qkv[:, D + g * DG:D + (g + 1) * DG]
        wv = W_qkv[:, 2 * D + g * DG:2 * D + (g + 1) * DG]
        wqk = np.concatenate([wq, wk], axis=1).astype(bf16)
        bq = b_qkv[g * DG:(g + 1) * DG]
        bk = b_qkv[D + g * DG:D + (g + 1) * DG]
        bqk = np.concatenate([bq, bk]).astype(np.float32)
        wo = W_out[g * DG:(g + 1) * DG, :].astype(bf16)
        in_maps.append({
            "xT": xT,
            "wqk": wqk,
            "wv": np.ascontiguousarray(wv).astype(bf16),
            "wo": np.ascontiguousarray(wo),
            "bqk": bqk,
            "masks": masks,
            "oh": oh,
        })
    return in_maps


def _np_fallback(x, W_qkv, b_qkv, W_out, b_out):
    out = np.empty((B, T, D), np.float32)
    qkv = x.reshape(B * T, D) @ W_qkv + b_qkv
    q, k, v = np.split(qkv.reshape(B, T, 3 * D), 3, axis=-1)

    def heads(z):
        return z.reshape(B, T, H, HD).transpose(0, 2, 1, 3)

    q, k, v = heads(q), heads(k), heads(v)
    causal = np.tril(np.ones((T, T), dtype=bool))
    acc = np.empty((B, H, T, HD), np.float32)
    for bi in range(B):
        for h in range(H):
            s = (q[bi, h] @ k[bi, h].T) * np.float32(SCALE)
            s = np.where(causal, s, -np.inf)
            s -= s.max(axis=-1, keepdims=True)
            p = np.exp(s)
            p /= p.sum(axis=-1, keepdims=True)
            acc[bi, h] = p @ v[bi, h]
    a = acc.transpose(0, 2, 1, 3).reshape(B, T, D)
    for bi in range(B):
        out[bi] = a[bi] @ W_out + b_out
    return out


def run(x, W_qkv, b_qkv, W_out, b_out, trace=False, trace_kwargs=None):
    from concourse import bass_utils

    x = np.asarray(x, np.float32)
    W_qkv = np.asarray(W_qkv, np.float32)
    b_qkv = np.asarray(b_qkv, np.float32)
    W_out = np.asarray(W_out, np.float32)
    b_out = np.asarray(b_out, np.float32)

    # the on-device kernel folds b_qkv's q/k slices in; its v slice is
    # assumed zero (true for this problem family). Fall back if not.
    if np.any(b_qkv[2 * D:]):
        return _np_fallback(x, W_qkv, b_qkv, W_out, b_out), None

    if "nc" not in _CACHE:
        _CACHE["nc"] = _build_bass()
    nc = _CACHE["nc"]

    in_maps = _make_in_maps(x, W_qkv, b_qkv, W_out)
    kw = dict(trace=trace)
    if trace_kwargs:
        kw.update(trace_kwargs)
    res = bass_utils.run_bass_kernel_spmd(nc, in_maps, list(range(NCORES)), **kw)

    out = np.empty((B, T, D), np.float32)
    for b in range(B):
        out[b] = (np.asarray(res.results[2 * b]["out"], np.float32)
                  + np.asarray(res.results[2 * b + 1]["out"], np.float32)
                  + b_out)
    return out, res


def kernel(x, W_qkv, b_qkv, W_out, b_out):
    out, _ = run(x, W_qkv, b_qkv, W_out, b_out, trace=False)
    return out


# revision 12
# speedup vs baseline: 1.2937x; 1.1811x over previous
"""Causal self-attention on 8 TRN2 NeuronCores.

Sharding: core c = (batch b = c // 2, head-group g = c % 2).
Each core handles one batch and 8 of the 16 heads:
  - QKV projection for its 512 q/k/v feature slices (transposed layout)
  - causal attention for its 8 heads
  - partial output projection (its 512 rows of W_out)
Host sums the two partials per batch and adds b_out.

All TensorE matmuls run in bf16; softmax runs in f32 (exp on ScalarE,
normalization via ones-matmul column sums + VectorE reciprocal).

PE array tiling:
  - scores: the two heads of a pair are row-tiled (K=64 lhsT at partitions
    0-63 / 64-127) and run concurrently in the top/bottom array halves.
  - AV: the two heads are col-tiled (M=64 outputs at psum partitions
    0-63 / 64-127) and run concurrently in the left/right array halves.
  - softmax denominators: M=1 ones-matmuls accumulate into four distinct
    32-col groups (head x kc-parity) of one psum bank, so consecutive
    chunks' denominator matmuls also overlap.
  - normalization: denominator reciprocals for the 8 heads live on 8 SBUF
    partitions; one K=8 one-hot matmul per head pair replicates them
    across 128 psum partitions for the VectorE multiply.
"""

import numpy as np
import ml_dtypes

B, T, D, H = 4, 2048, 1024, 16
HG = 2            # head groups (tensor-parallel factor)
HL = H // HG      # 8 heads per core
HD = D // H       # 64
DG = HL * HD      # 512 features per group
SCALE = 1.0 / float(np.sqrt(HD))
NCORES = 8
TCH = T // 128    # 16 time chunks of 128
NQC = T // 512    # 4 query chunks of 512

bf16 = ml_dtypes.bfloat16

_CACHE = {}


def _split_multi_waits(nc, mybir):
    """The TPB instruction encoding has a single wait slot; this walrus build
    rejects instructions carrying more than one sync wait. Hoist extra waits
    onto standalone EventSemaphore instructions on the same engine. Tile's
    schedule is a valid serialization (waits only reference earlier-ordered
    work on other streams), so blocking the issuing stream at the same point
    cannot deadlock."""
    SKIP = ("InstTriggerDma", "InstCollectiveCompute")
    for f in nc.m.functions:
        for blk in f.blocks:
            out = []
            changed = False
            for inst in blk.instructions:
                si = getattr(inst, "sync_info", None)
                ow = list(si.on_wait) if si is not None and si.on_wait else []
                if len(ow) > 1 and type(inst).__name__ not in SKIP:
                    for i, w in enumerate(ow[:-1]):
                        out.append(mybir.InstEventSemaphore(
                            name=f"{inst.name}_hw{i}",
                            engine=inst.engine,
                            sync_info=mybir.SyncInfo(on_wait=[w], on_update=[]),
                            bass_nofuse=True,
                        ))
                    inst.sync_info = mybir.SyncInfo(
                        on_wait=[ow[-1]],
                        on_update=list(si.on_update) if si.on_update else [],
                    )
                    changed = True
                out.append(inst)
            if changed:
                blk.instructions = out


def _build_bass():
    import concourse.bass as bass
    import concourse.mybir as mybir
    import concourse.tile as tile
    from contextlib import ExitStack

    dt = mybir.dt
    f32 = dt.float32
    bf = dt.bfloat16

    nc = bass.Bass()
    xT_d = nc.declare_dram_parameter("xT", [D, T], bf, isOutput=False)
    wqk_d = nc.declare_dram_parameter("wqk", [D, 2 * DG], bf, isOutput=False)
    wv_d = nc.declare_dram_parameter("wv", [D, DG], bf, isOutput=False)
    wo_d = nc.declare_dram_parameter("wo", [DG, D], bf, isOutput=False)
    bqk_d = nc.declare_dram_parameter("bqk", [2 * DG], f32, isOutput=False)
    masks_d = nc.declare_dram_parameter("masks", [128, 4096], bf, isOutput=False)
    oh_d = nc.declare_dram_parameter("oh", [8, 4 * 128], bf, isOutput=False)
    out_d = nc.declare_dram_parameter("out", [T, D], f32, isOutput=True)

    with tile.TileContext(nc) as tc, ExitStack() as ctx:
        const = ctx.enter_context(tc.tile_pool(name="const", bufs=1))
        psum = ctx.enter_context(tc.tile_pool(name="psum", bufs=2, space="PSUM"))
        ptp = ctx.enter_context(tc.tile_pool(name="ptp", bufs=5))
        stp = ctx.enter_context(tc.tile_pool(name="stp", bufs=10))
        small = ctx.enter_context(tc.tile_pool(name="small", bufs=3))

        # ---- resident tensors --------------------------------------------
        xT_sb = const.tile([128, 8, T], bf)          # x[b].T   (feature-major)
        wqk_sb = const.tile([128, 8, 2 * DG], bf)    # W_qkv q|k columns
        wv_sb = const.tile([128, 8, DG], bf)         # W_qkv v columns
        wo_sb = const.tile([128, 4, D], bf)          # W_out rows for group
        qkT_sb = const.tile([128, 8, T], bf)         # [q^T | k^T]  (feature-major)
        vn_sb = const.tile([128, TCH, DG], bf)       # V (key-major, 8 heads x 64)
        at_sb = const.tile([128, 4, T], bf)          # A^T (normalized attn out)
        masks_sb = const.tile([128, 4096], bf)       # per-dg diagonal masks x2 heads
        bqk_sb = const.tile([128, 8], f32)
        oh_sb = const.tile([8, 4 * 128], bf)         # K=8 one-hot lhsT per pair
        onecol = const.tile([128, 1], bf)            # all-ones stationary column

        nc.vector.memset(onecol, 1.0)

        # initial loads: 3 queues; xT comes in token slices so the first
        # matmuls only wait for ~2MB, not the full 9.5MB working set
        for s0, s1 in ((0, 512), (512, 1024), (1024, 2048)):
            for c in range(8):
                nc.sync.dma_start(
                    out=xT_sb[:, c, s0:s1],
                    in_=xT_d[c * 128:(c + 1) * 128, s0:s1])
        for c in range(8):
            nc.gpsimd.dma_start(out=wv_sb[:, c, :], in_=wv_d[c * 128:(c + 1) * 128, :])
        for c in range(8):
            nc.gpsimd.dma_start(out=wqk_sb[:, c, :], in_=wqk_d[c * 128:(c + 1) * 128, :])
        nc.scalar.dma_start(out=bqk_sb, in_=bqk_d[:].rearrange("(c p) -> p c", p=128))
        nc.scalar.dma_start(out=oh_sb, in_=oh_d[:, :])
        nc.scalar.dma_start(out=masks_sb, in_=masks_d[:, :])
        for c in range(4):
            nc.scalar.dma_start(out=wo_sb[:, c, :], in_=wo_d[c * 128:(c + 1) * 128, :])

        def qkv_v_chunk(tn):
            pv = psum.tile([128, 512], f32, tag="mm512", name=f"pv{tn}")
            for k in range(8):
                nc.tensor.matmul(
                    pv,
                    lhsT=xT_sb[:, k, tn * 128:(tn + 1) * 128],
                    rhs=wv_sb[:, k, :],
                    start=(k == 0), stop=(k == 7),
                )
            nc.vector.tensor_copy(out=vn_sb[:, tn, :], in_=pv)

        def qkv_qk_chunk(m, n):
            """Produce qkT features m*128..m*128+128 for token slice n."""
            pq = psum.tile([128, 512], f32, tag="mm512", name=f"pq{m}_{n}")
            for k in range(8):
                nc.tensor.matmul(
                    pq,
                    lhsT=wqk_sb[:, k, m * 128:(m + 1) * 128],
                    rhs=xT_sb[:, k, n * 512:(n + 1) * 512],
                    start=(k == 0), stop=(k == 7),
                )
            # psum -> SBUF bf16 with per-partition bias add, on VectorE so the
            # ScalarE stays dedicated to softmax exps
            nc.vector.tensor_scalar_add(
                out=qkT_sb[:, m, n * 512:(n + 1) * 512],
                in0=pq, scalar1=bqk_sb[:, m:m + 1],
            )

        # ---- attention (interleaved with QKV production) -----------------
        stages = {}
        ge8 = {}
        go8 = {}

        def attn(qc, p, fill=None):
            """Scores + AV + denominators for head pair p of query chunk qc.

            Each score group is one kc for both heads ([128, 1024] psum, two
            concurrent row-tiled K=64 matmuls); ScalarE exps it into pt.
            AV (col-tiled pair) and denominator matmuls for groups g-2, g-1
            are emitted after group g's score matmuls so the PE stream always
            has work while exp runs."""
            nkc = 4 * qc + 4
            h0, h1 = 2 * p, 2 * p + 1
            if p == 0:
                ge8[qc] = stp.tile([8, 512], f32, tag="ge8", bufs=2,
                                   name=f"ge8_{qc}")
                go8[qc] = stp.tile([8, 512], f32, tag="go8", bufs=2,
                                   name=f"go8_{qc}")
            qsl0 = qkT_sb[0:64, p, qc * 512:(qc + 1) * 512]
            qsl1 = qkT_sb[64:128, p, qc * 512:(qc + 1) * 512]
            pts = []
            ptsums = []
            pav = psum.tile([128, 512], f32, tag="av", name=f"pav{qc}_{p}")
            pones = psum.tile([128, 512], f32, tag="av", name=f"pones{qc}_{p}")

            def av_group(kc):
                dg = kc - (nkc - 4)
                lo = 128 * dg if dg >= 1 else 0
                nc.tensor.matmul(
                    pav[0:64, lo:512],
                    lhsT=vn_sb[:, kc, h0 * HD:(h0 + 1) * HD],
                    rhs=pts[kc][:, lo:512],
                    start=(kc == 0), stop=(kc == nkc - 1),
                )
                nc.tensor.matmul(
                    pav[64:128, lo:512],
                    lhsT=vn_sb[:, kc, h1 * HD:(h1 + 1) * HD],
                    rhs=pts[kc][:, 512 + lo:1024],
                    start=(kc == 0), stop=(kc == nkc - 1),
                )

            def ones_pair(j):
                # denominator matmuls over the VectorE-precomputed pt pair
                # sum; head x pair-parity -> 4 distinct col groups (psum rows
                # 0/32/64/96) so consecutive pairs' matmuls overlap
                np_ = nkc // 2
                j0, j1 = j % 2, 2 + j % 2
                nc.tensor.matmul(
                    pones[32 * j0:32 * j0 + 1, :],
                    lhsT=onecol, rhs=ptsums[j][:, 0:512],
                    start=(j < 2), stop=(j >= np_ - 2),
                    tile_position=(0, 32 * j0),
                )
                nc.tensor.matmul(
                    pones[32 * j1:32 * j1 + 1, :],
                    lhsT=onecol, rhs=ptsums[j][:, 512:1024],
                    start=(j < 2), stop=(j >= np_ - 2),
                    tile_position=(0, 32 * j1),
                )

            def batch(kc):
                ptsum = ptp.tile([128, 1024], bf, tag="ptsum", bufs=3,
                                 name=f"pts{qc}_{p}_{kc}")
                nc.vector.tensor_add(out=ptsum, in0=pts[kc], in1=pts[kc + 1])
                ptsums.append(ptsum)
                av_group(kc)
                av_group(kc + 1)
                if kc >= 2:
                    # trail by one batch so the PE never waits on VectorE
                    ones_pair(kc // 2 - 1)
                if fill:
                    fill.pop(0)()

            for kc in range(nkc):
                dg = kc - (nkc - 4)  # 0..3 on the masked diagonal band
                slo = 128 * dg if dg >= 1 else 0
                ps = psum.tile([128, 1024], f32, tag="s", name=f"ps{qc}_{p}_{kc}")
                nc.tensor.matmul(
                    ps[:, slo:512],
                    lhsT=qkT_sb[0:64, 4 + p, kc * 128:(kc + 1) * 128],
                    rhs=qsl0[:, slo:512], start=True, stop=True,
                )
                nc.tensor.matmul(
                    ps[:, 512 + slo:1024],
                    lhsT=qkT_sb[64:128, 4 + p, kc * 128:(kc + 1) * 128],
                    rhs=qsl1[:, slo:512], start=True, stop=True,
                )
                if kc > 0 and kc % 2 == 0:
                    batch(kc - 2)
                pt = ptp.tile([128, 1024], bf, tag="pt", name=f"pt{qc}_{p}_{kc}")
                ptv = pt.rearrange("p (h c) -> p h c", c=512)
                psv = ps.rearrange("p (h c) -> p h c", c=512)
                if dg >= 1:
                    # diagonal tiles: exp only the causally-live columns
                    lo = 128 * dg
                    nc.vector.memset(ptv[:, :, 0:lo], 0.0)
                    nc.scalar.activation(
                        out=ptv[:, :, lo:512], in_=psv[:, :, lo:512],
                        func=mybir.ActivationFunctionType.Exp, scale=SCALE,
                    )
                else:
                    nc.scalar.activation(
                        out=pt, in_=ps,
                        func=mybir.ActivationFunctionType.Exp, scale=SCALE,
                    )
                if dg >= 0:
                    # only 128 columns per head straddle the diagonal; the
                    # rest are fully live (or already memset to zero)
                    lo = 128 * dg
                    mv = masks_sb[:, dg * 1024:(dg + 1) * 1024].rearrange(
                        "p (h c) -> p h c", c=512)
                    nc.vector.tensor_mul(
                        out=ptv[:, :, lo:lo + 128], in0=ptv[:, :, lo:lo + 128],
                        in1=mv[:, :, lo:lo + 128],
                    )
                pts.append(pt)
            batch(nkc - 2)
            ones_pair(nkc // 2 - 1)

            stage = stp.tile([128, 512], bf, tag="stage", bufs=6,
                             name=f"st{qc}_{p}")
            nc.vector.tensor_copy(out=stage, in_=pav)
            stages[(qc, p)] = stage
            # denominator partials (4 col-group rows) -> SBUF, then gather the
            # strided rows into head-major order for the VectorE add
            pone_sb = stp.tile([97, 512], f32, tag="pone", bufs=2,
                               name=f"pone{qc}_{p}")
            nc.vector.tensor_copy(out=pone_sb, in_=pones[0:97, :])
            nc.gpsimd.dma_start(out=ge8[qc][2 * p:2 * p + 1, :], in_=pone_sb[0:1, :])
            nc.gpsimd.dma_start(out=go8[qc][2 * p:2 * p + 1, :], in_=pone_sb[32:33, :])
            nc.gpsimd.dma_start(out=ge8[qc][2 * p + 1:2 * p + 2, :], in_=pone_sb[64:65, :])
            nc.gpsimd.dma_start(out=go8[qc][2 * p + 1:2 * p + 2, :], in_=pone_sb[96:97, :])

        def divisions(qc):
            coll = stp.tile([8, 512], f32, tag="coll", bufs=2, name=f"coll{qc}")
            nc.vector.tensor_add(out=coll, in0=ge8[qc], in1=go8[qc])
            rcoll = stp.tile([8, 512], f32, tag="rcoll", bufs=2)
            nc.vector.reciprocal(rcoll, coll)
            rcollb = stp.tile([8, 512], bf, tag="rcollb", bufs=2)
            nc.vector.tensor_copy(out=rcollb, in_=rcoll)
            for p in range(4):
                # replicate rows (2p, 2p+1) of rcollb across 64 psum
                # partitions each via a K=8 one-hot stationary matmul
                prb = psum.tile([128, 512], f32, tag="mm512", name=f"prb{qc}_{p}")
                nc.tensor.matmul(
                    prb, lhsT=oh_sb[:, p * 128:(p + 1) * 128], rhs=rcollb,
                    start=True, stop=True,
                )
                nc.vector.tensor_mul(
                    out=at_sb[:, p, qc * 512:(qc + 1) * 512],
                    in0=stages[(qc, p)], in1=prb,
                )

        def outproj_unit(qj, dn):
            def emit():
                po = psum.tile([128, 512], f32, tag="mm512",
                               name=f"po{qj}_{dn}")
                for kc in range(4):
                    nc.tensor.matmul(
                        po,
                        lhsT=at_sb[:, kc, qj * 128:(qj + 1) * 128],
                        rhs=wo_sb[:, kc, dn * 512:(dn + 1) * 512],
                        start=(kc == 0), stop=(kc == 3),
                    )
                ost = small.tile([128, 512], f32, tag="ost")
                nc.vector.tensor_copy(out=ost, in_=po)
                nc.sync.dma_start(
                    out=out_d[qj * 128:(qj + 1) * 128,
                              dn * 512:(dn + 1) * 512],
                    in_=ost,
                )
            return emit

        def outproj_units(qc):
            return [outproj_unit(qj, dn)
                    for qj in range(4 * qc, 4 * qc + 4) for dn in range(2)]

        # qkT production is sliced by token chunk n and emitted just ahead of
        # the attention block that consumes it, so ScalarE exp work starts
        # early and paces the whole kernel; spare PE slots inside attention
        # blocks are filled from a unit queue (V chunks, outproj, leftovers).
        for tn in range(4):
            qkv_v_chunk(tn)
        fq = [(lambda t: (lambda: qkv_v_chunk(t)))(tn) for tn in range(4, 8)]
        for p in range(4):
            qkv_qk_chunk(p, 0)
            qkv_qk_chunk(4 + p, 0)
            attn(0, p, fq)
        for u in fq:
            u()
        fq = [(lambda t: (lambda: qkv_v_chunk(t)))(tn) for tn in range(8, 12)]
        for p in range(4):
            qkv_qk_chunk(p, 1)
            qkv_qk_chunk(4 + p, 1)
            attn(1, p, fq)
            if p == 1:
                divisions(0)
        for u in fq:
            u()
        fq = [(lambda t: (lambda: qkv_v_chunk(t)))(tn) for tn in range(12, 16)]
        fq += outproj_units(0)
        for p in range(4):
            qkv_qk_chunk(p, 2)
            qkv_qk_chunk(4 + p, 2)
            attn(2, p, fq)
            if p == 1:
                divisions(1)
        for u in fq[:4]:
            u()
        fq = fq[4:] + outproj_units(1)
        for p in range(4):
            qkv_qk_chunk(p, 3)
            qkv_qk_chunk(4 + p, 3)
            attn(3, p, fq)
            if p == 1:
                divisions(2)
            if p == 2:
                fq += outproj_units(2)
        divisions(3)
        for u in fq:
            u()
        for u in outproj_units(3):
            u()

    _split_multi_waits(nc, mybir)
    return nc


def _make_masks():
    kl = np.arange(128)[:, None]
    ql = np.arange(512)[None, :]
    t = [(ql >= kl + 128 * i).astype(np.float32) for i in range(4)]
    # block dg holds the mask for diagonal offset 128*dg, duplicated for the
    # two heads packed side by side in each [128, 1024] score group
    return np.concatenate([np.concatenate([m, m], axis=1) for m in t],
                          axis=1).astype(bf16)  # [128, 4096]


def _make_in_maps(x, W_qkv, b_qkv, W_out):
    masks = _make_masks()
    # oh[k, 128p + c] = (k == 2p + (c >= 64)): K=8 one-hot stationary that
    # replicates head-pair p's two reciprocal rows across 128 psum partitions
    oh = np.zeros((8, 4, 128), np.float32)
    for p in range(4):
        oh[2 * p, p, 0:64] = 1.0
        oh[2 * p + 1, p, 64:128] = 1.0
    oh = oh.reshape(8, 4 * 128).astype(bf16)
    in_maps = []
    for c in range(NCORES):
        b, g = divmod(c, 2)
        xT = np.ascontiguousarray(x[b].T).astype(bf16)
        wq = W_qkv[:, g * DG:(g + 1) * DG]
        wk = W_qkv[:, D + g * DG:D + (g + 1) * DG]
        wv = W_qkv[:, 2 * D + g * DG:2 * D + (g + 1) * DG]
        wqk = np.concatenate([wq, wk], axis=1).astype(bf16)
        bq = b_qkv[g * DG:(g + 1) * DG]
        bk = b_qkv[D + g * DG:D + (g + 1) * DG]
        bqk = np.concatenate([bq, bk]).astype(np.float32)
        wo = W_out[g * DG:(g + 1) * DG, :].astype(bf16)
        in_maps.append({
            "xT": xT,
            "wqk": wqk,
            "wv": np.ascontiguousarray(wv).astype(bf16),
            "wo": np.ascontiguousarray(wo),
            "bqk": bqk,
            "masks": masks,
            "oh": oh,
        })
    return in_maps


def _np_fallback(x, W_qkv, b_qkv, W_out, b_out):
    out = np.empty((B, T, D), np.float32)
    qkv = x.reshape(B * T, D) @ W_qkv + b_qkv
    q, k, v = np.split(qkv.reshape(B, T, 3 * D), 3, axis=-1)

    def heads(z):
        return z.reshape(B, T, H, HD).transpose(0, 2, 1, 3)

    q, k, v = heads(q), heads(k), heads(v)
    causal = np.tril(np.ones((T, T), dtype=bool))
    acc = np.empty((B, H, T, HD), np.float32)
    for bi in range(B):
        for h in range(H):
            s = (q[bi, h] @ k[bi, h].T) * np.float32(SCALE)
            s = np.where(causal, s, -np.inf)
            s -= s.max(axis=-1, keepdims=True)
            p = np.exp(s)
            p /= p.sum(axis=-1, keepdims=True)
            acc[bi, h] = p @ v[bi, h]
    a = acc.transpose(0, 2, 1, 3).reshape(B, T, D)
    for bi in range(B):
        out[bi] = a[bi] @ W_out + b_out
    return out


def run(x, W_qkv, b_qkv, W_out, b_out, trace=False, trace_kwargs=None):
    from concourse import bass_utils

    x = np.asarray(x, np.float32)
    W_qkv = np.asarray(W_qkv, np.float32)
    b_qkv = np.asarray(b_qkv, np.float32)
    W_out = np.asarray(W_out, np.float32)
    b_out = np.asarray(b_out, np.float32)

    # the on-device kernel folds b_qkv's q/k slices in; its v slice is
    # assumed zero (true for this problem family). Fall back if not.
    if np.any(b_qkv[2 * D:]):
        return _np_fallback(x, W_qkv, b_qkv, W_out, b_out), None

    if "nc" not in _CACHE:
        _CACHE["nc"] = _build_bass()
    nc = _CACHE["nc"]

    in_maps = _make_in_maps(x, W_qkv, b_qkv, W_out)
    kw = dict(trace=trace)
    if trace_kwargs:
        kw.update(trace_kwargs)
    res = bass_utils.run_bass_kernel_spmd(nc, in_maps, list(range(NCORES)), **kw)

    out = np.empty((B, T, D), np.float32)
    for b in range(B):
        out[b] = (np.asarray(res.results[2 * b]["out"], np.float32)
                  + np.asarray(res.results[2 * b + 1]["out"], np.float32)
                  + b_out)
    return out, res


def kernel(x, W_qkv, b_qkv, W_out, b_out):
    out, _ = run(x, W_qkv, b_qkv, W_out, b_out, trace=False)
    return out


# revision 18
# speedup vs baseline: 1.3033x; 1.0075x over previous
"""Causal self-attention on 8 TRN2 NeuronCores.

Sharding: core c = (batch b = c // 2, head-group g = c % 2).
Each core handles one batch and 8 of the 16 heads:
  - QKV projection for its 512 q/k/v feature slices (transposed layout)
  - causal attention for its 8 heads
  - partial output projection (its 512 rows of W_out)
Host sums the two partials per batch and adds b_out.

All TensorE matmuls run in bf16; softmax runs in f32 (exp on ScalarE,
normalization via ones-matmul column sums + VectorE reciprocal).

PE array tiling:
  - scores: the two heads of a pair are row-tiled (K=64 lhsT at partitions
    0-63 / 64-127) and run concurrently in the top/bottom array halves.
  - AV: the two heads are col-tiled (M=64 outputs at psum partitions
    0-63 / 64-127) and run concurrently in the left/right array halves.
  - softmax denominators: M=1 ones-matmuls accumulate into four distinct
    32-col groups (head x kc-parity) of one psum bank, so consecutive
    chunks' denominator matmuls also overlap.
  - normalization: denominator reciprocals for the 8 heads live on 8 SBUF
    partitions; one K=8 one-hot matmul per head pair replicates them
    across 128 psum partitions for the VectorE multiply.
"""

import numpy as np
import ml_dtypes

B, T, D, H = 4, 2048, 1024, 16
HG = 2            # head groups (tensor-parallel factor)
HL = H // HG      # 8 heads per core
HD = D // H       # 64
DG = HL * HD      # 512 features per group
SCALE = 1.0 / float(np.sqrt(HD))
NCORES = 8
TCH = T // 128    # 16 time chunks of 128
NQC = T // 512    # 4 query chunks of 512

bf16 = ml_dtypes.bfloat16

_CACHE = {}


def _split_multi_waits(nc, mybir):
    """The TPB instruction encoding has a single wait slot; this walrus build
    rejects instructions carrying more than one sync wait. Hoist extra waits
    onto standalone EventSemaphore instructions on the same engine. Tile's
    schedule is a valid serialization (waits only reference earlier-ordered
    work on other streams), so blocking the issuing stream at the same point
    cannot deadlock."""
    SKIP = ("InstTriggerDma", "InstCollectiveCompute")
    for f in nc.m.functions:
        for blk in f.blocks:
            out = []
            changed = False
            for inst in blk.instructions:
                si = getattr(inst, "sync_info", None)
                ow = list(si.on_wait) if si is not None and si.on_wait else []
                if len(ow) > 1 and type(inst).__name__ not in SKIP:
                    for i, w in enumerate(ow[:-1]):
                        out.append(mybir.InstEventSemaphore(
                            name=f"{inst.name}_hw{i}",
                            engine=inst.engine,
                            sync_info=mybir.SyncInfo(on_wait=[w], on_update=[]),
                            bass_nofuse=True,
                        ))
                    inst.sync_info = mybir.SyncInfo(
                        on_wait=[ow[-1]],
                        on_update=list(si.on_update) if si.on_update else [],
                    )
                    changed = True
                out.append(inst)
            if changed:
                blk.instructions = out


def _build_bass():
    import concourse.bass as bass
    import concourse.mybir as mybir
    import concourse.tile as tile
    from contextlib import ExitStack

    dt = mybir.dt
    f32 = dt.float32
    bf = dt.bfloat16

    nc = bass.Bass()
    xT_d = nc.declare_dram_parameter("xT", [D, T], bf, isOutput=False)
    wqk_d = nc.declare_dram_parameter("wqk", [D, 2 * DG], bf, isOutput=False)
    wv_d = nc.declare_dram_parameter("wv", [D, DG], bf, isOutput=False)
    wo_d = nc.declare_dram_parameter("wo", [DG, D], bf, isOutput=False)
    bqk_d = nc.declare_dram_parameter("bqk", [2 * DG], f32, isOutput=False)
    masks_d = nc.declare_dram_parameter("masks", [128, 4096], bf, isOutput=False)
    oh_d = nc.declare_dram_parameter("oh", [32, 16 * 128], bf, isOutput=False)
    out_d = nc.declare_dram_parameter("out", [T, D], f32, isOutput=True)

    with tile.TileContext(nc) as tc, ExitStack() as ctx:
        const = ctx.enter_context(tc.tile_pool(name="const", bufs=1))
        psum = ctx.enter_context(tc.tile_pool(name="psum", bufs=2, space="PSUM"))
        ptp = ctx.enter_context(tc.tile_pool(name="ptp", bufs=5))
        stp = ctx.enter_context(tc.tile_pool(name="stp", bufs=10))
        small = ctx.enter_context(tc.tile_pool(name="small", bufs=3))

        # ---- resident tensors --------------------------------------------
        xT_sb = const.tile([128, 8, T], bf)          # x[b].T   (feature-major)
        wqk_sb = const.tile([128, 8, 2 * DG], bf)    # W_qkv q|k columns
        wv_sb = const.tile([128, 8, DG], bf)         # W_qkv v columns
        wo_sb = const.tile([128, 4, D], bf)          # W_out rows for group
        qkT_sb = const.tile([128, 8, T], bf)         # [q^T | k^T]  (feature-major)
        vn_sb = const.tile([128, TCH, DG], bf)       # V (key-major, 8 heads x 64)
        at_sb = const.tile([128, 4, T], bf)          # A^T (normalized attn out)
        masks_sb = const.tile([128, 4096], bf)       # per-dg diagonal masks x2 heads
        bqk_sb = const.tile([128, 8], f32)
        oh_sb = const.tile([32, 16 * 128], bf)       # K=32 one-hot lhsT per (pair, seg)
        onecol = const.tile([128, 1], bf)            # all-ones stationary column

        nc.vector.memset(onecol, 1.0)

        # initial loads, striped across three DMA queues in dependency order:
        # the first V/QK matmuls wait on ~4MB, not the full 9.5MB working set
        queues = [nc.sync, nc.gpsimd, nc.scalar]
        group_a = ([(xT_sb[:, c, 0:512], xT_d[c * 128:(c + 1) * 128, 0:512])
                    for c in range(8)]
                   + [(wv_sb[:, c, :], wv_d[c * 128:(c + 1) * 128, :])
                      for c in range(8)])
        group_b = ([(wqk_sb[:, c, :], wqk_d[c * 128:(c + 1) * 128, :])
                    for c in range(8)]
                   + [(bqk_sb, bqk_d[:].rearrange("(c p) -> p c", p=128)),
                      (oh_sb, oh_d[:, :]),
                      (masks_sb, masks_d[:, :])]
                   + [(xT_sb[:, c, 512:1024], xT_d[c * 128:(c + 1) * 128, 512:1024])
                      for c in range(8)]
                   + [(xT_sb[:, c, 1024:2048], xT_d[c * 128:(c + 1) * 128, 1024:2048])
                      for c in range(8)]
                   + [(wo_sb[:, c, :], wo_d[c * 128:(c + 1) * 128, :])
                      for c in range(4)])
        for i, (dst, src) in enumerate(group_a + group_b):
            queues[i % 3].dma_start(out=dst, in_=src)

        def qkv_v_chunk(tn):
            pv = psum.tile([128, 512], f32, tag="mm512", name=f"pv{tn}")
            for k in range(8):
                nc.tensor.matmul(
                    pv,
                    lhsT=xT_sb[:, k, tn * 128:(tn + 1) * 128],
                    rhs=wv_sb[:, k, :],
                    start=(k == 0), stop=(k == 7),
                )
            nc.vector.tensor_copy(out=vn_sb[:, tn, :], in_=pv)

        def qkv_qk_chunk(m, n):
            """Produce qkT features m*128..m*128+128 for token slice n."""
            pq = psum.tile([128, 512], f32, tag="mm512", name=f"pq{m}_{n}")
            for k in range(8):
                nc.tensor.matmul(
                    pq,
                    lhsT=wqk_sb[:, k, m * 128:(m + 1) * 128],
                    rhs=xT_sb[:, k, n * 512:(n + 1) * 512],
                    start=(k == 0), stop=(k == 7),
                )
            # psum -> SBUF bf16 with per-partition bias add, on VectorE so the
            # ScalarE stays dedicated to softmax exps
            nc.vector.tensor_scalar_add(
                out=qkT_sb[:, m, n * 512:(n + 1) * 512],
                in0=pq, scalar1=bqk_sb[:, m:m + 1],
            )

        # ---- attention (interleaved with QKV production) -----------------
        stages = {}
        ge8 = {}
        go8 = {}

        def attn(qc, p, fill=None):
            """Scores + AV + denominators for head pair p of query chunk qc.

            Each score group is one kc for both heads ([128, 1024] psum, two
            concurrent row-tiled K=64 matmuls); ScalarE exps it into pt.
            AV (col-tiled pair) and denominator matmuls for groups g-2, g-1
            are emitted after group g's score matmuls so the PE stream always
            has work while exp runs."""
            nkc = 4 * qc + 4
            h0, h1 = 2 * p, 2 * p + 1
            if p == 0:
                ge8[qc] = stp.tile([32, 128], f32, tag="ge8", bufs=2,
                                   name=f"ge8_{qc}")
                go8[qc] = stp.tile([32, 128], f32, tag="go8", bufs=2,
                                   name=f"go8_{qc}")
            qsl0 = qkT_sb[0:64, p, qc * 512:(qc + 1) * 512]
            qsl1 = qkT_sb[64:128, p, qc * 512:(qc + 1) * 512]
            pts = []
            ptsums = []
            pav = psum.tile([128, 512], f32, tag="av", name=f"pav{qc}_{p}")
            pones = psum.tile([128, 512], f32, tag="av", name=f"pones{qc}_{p}")

            def av_group(kc):
                dg = kc - (nkc - 4)
                lo = 128 * dg if dg >= 1 else 0
                nc.tensor.matmul(
                    pav[0:64, lo:512],
                    lhsT=vn_sb[:, kc, h0 * HD:(h0 + 1) * HD],
                    rhs=pts[kc][:, lo:512],
                    start=(kc == 0), stop=(kc == nkc - 1),
                )
                nc.tensor.matmul(
                    pav[64:128, lo:512],
                    lhsT=vn_sb[:, kc, h1 * HD:(h1 + 1) * HD],
                    rhs=pts[kc][:, 512 + lo:1024],
                    start=(kc == 0), stop=(kc == nkc - 1),
                )

            def ones_pair(j):
                # denominator matmuls over the VectorE-precomputed pt pair
                # sum; head x pair-parity -> 4 distinct col groups (psum rows
                # 0/32/64/96) so consecutive pairs' matmuls overlap
                np_ = nkc // 2
                j0, j1 = j % 2, 2 + j % 2
                nc.tensor.matmul(
                    pones[32 * j0:32 * j0 + 1, :],
                    lhsT=onecol, rhs=ptsums[j][:, 0:512],
                    start=(j < 2), stop=(j >= np_ - 2),
                    tile_position=(0, 32 * j0),
                )
                nc.tensor.matmul(
                    pones[32 * j1:32 * j1 + 1, :],
                    lhsT=onecol, rhs=ptsums[j][:, 512:1024],
                    start=(j < 2), stop=(j >= np_ - 2),
                    tile_position=(0, 32 * j1),
                )

            def batch(kc):
                ptsum = ptp.tile([128, 1024], bf, tag="ptsum", bufs=3,
                                 name=f"pts{qc}_{p}_{kc}")
                nc.vector.tensor_add(out=ptsum, in0=pts[kc], in1=pts[kc + 1])
                ptsums.append(ptsum)
                av_group(kc)
                av_group(kc + 1)
                if kc >= 2:
                    # trail by one batch so the PE never waits on VectorE
                    ones_pair(kc // 2 - 1)
                if fill:
                    fill.pop(0)()

            for kc in range(nkc):
                dg = kc - (nkc - 4)  # 0..3 on the masked diagonal band
                slo = 128 * dg if dg >= 1 else 0
                ps = psum.tile([128, 1024], f32, tag="s", name=f"ps{qc}_{p}_{kc}")
                nc.tensor.matmul(
                    ps[:, slo:512],
                    lhsT=qkT_sb[0:64, 4 + p, kc * 128:(kc + 1) * 128],
                    rhs=qsl0[:, slo:512], start=True, stop=True,
                )
                nc.tensor.matmul(
                    ps[:, 512 + slo:1024],
                    lhsT=qkT_sb[64:128, 4 + p, kc * 128:(kc + 1) * 128],
                    rhs=qsl1[:, slo:512], start=True, stop=True,
                )
                if kc > 0 and kc % 2 == 0:
                    batch(kc - 2)
                pt = ptp.tile([128, 1024], bf, tag="pt", name=f"pt{qc}_{p}_{kc}")
                ptv = pt.rearrange("p (h c) -> p h c", c=512)
                psv = ps.rearrange("p (h c) -> p h c", c=512)
                if dg >= 1:
                    # diagonal tiles: exp only the causally-live columns
                    lo = 128 * dg
                    nc.vector.memset(ptv[:, :, 0:lo], 0.0)
                    nc.scalar.activation(
                        out=ptv[:, :, lo:512], in_=psv[:, :, lo:512],
                        func=mybir.ActivationFunctionType.Exp, scale=SCALE,
                    )
                else:
                    nc.scalar.activation(
                        out=pt, in_=ps,
                        func=mybir.ActivationFunctionType.Exp, scale=SCALE,
                    )
                if dg >= 0:
                    # only 128 columns per head straddle the diagonal; the
                    # rest are fully live (or already memset to zero)
                    lo = 128 * dg
                    mv = masks_sb[:, dg * 1024:(dg + 1) * 1024].rearrange(
                        "p (h c) -> p h c", c=512)
                    nc.vector.tensor_mul(
                        out=ptv[:, :, lo:lo + 128], in0=ptv[:, :, lo:lo + 128],
                        in1=mv[:, :, lo:lo + 128],
                    )
                pts.append(pt)
            batch(nkc - 2)
            ones_pair(nkc // 2 - 1)

            stage = stp.tile([128, 512], bf, tag="stage", bufs=6,
                             name=f"st{qc}_{p}")
            nc.vector.tensor_copy(out=stage, in_=pav)
            stages[(qc, p)] = stage
            # denominator partials (4 col-group rows) -> SBUF, then gather the
            # strided rows as [4, 128] blocks so the add/reciprocal run wide
            pone_sb = stp.tile([97, 512], f32, tag="pone", bufs=2,
                               name=f"pone{qc}_{p}")
            nc.vector.tensor_copy(out=pone_sb, in_=pones[0:97, :])
            for r, buf, q in ((0, ge8, nc.gpsimd), (32, go8, nc.gpsimd),
                              (64, ge8, nc.sync), (96, go8, nc.sync)):
                h = (r // 64) & 1
                q.dma_start(
                    out=buf[qc][8 * p + 4 * h:8 * p + 4 * h + 4, :],
                    in_=pone_sb[r:r + 1, :].rearrange("o (a b) -> o a b", b=128),
                )

        def divisions(qc):
            coll = stp.tile([32, 128], f32, tag="coll", bufs=2, name=f"coll{qc}")
            nc.vector.tensor_add(out=coll, in0=ge8[qc], in1=go8[qc])
            rcoll = stp.tile([32, 128], f32, tag="rcoll", bufs=2)
            nc.vector.reciprocal(rcoll, coll)
            rcollb = stp.tile([32, 128], bf, tag="rcollb", bufs=2)
            nc.vector.tensor_copy(out=rcollb, in_=rcoll)
            for p in range(4):
                # replicate rows 8p..8p+8 of rcollb across 128 psum
                # partitions via K=32 one-hot stationary matmuls (one per
                # 128-query segment)
                prb = psum.tile([128, 512], f32, tag="mm512", name=f"prb{qc}_{p}")
                for a in range(4):
                    nc.tensor.matmul(
                        prb[:, a * 128:(a + 1) * 128],
                        lhsT=oh_sb[:, (p * 4 + a) * 128:(p * 4 + a + 1) * 128],
                        rhs=rcollb, start=True, stop=True,
                    )
                nc.vector.tensor_mul(
                    out=at_sb[:, p, qc * 512:(qc + 1) * 512],
                    in0=stages[(qc, p)], in1=prb,
                )

        def outproj_unit(qj, dn):
            def emit():
                po = psum.tile([128, 512], f32, tag="mm512",
                               name=f"po{qj}_{dn}")
                for kc in range(4):
                    nc.tensor.matmul(
                        po,
                        lhsT=at_sb[:, kc, qj * 128:(qj + 1) * 128],
                        rhs=wo_sb[:, kc, dn * 512:(dn + 1) * 512],
                        start=(kc == 0), stop=(kc == 3),
                    )
                ost = small.tile([128, 512], f32, tag="ost")
                nc.vector.tensor_copy(out=ost, in_=po)
                nc.sync.dma_start(
                    out=out_d[qj * 128:(qj + 1) * 128,
                              dn * 512:(dn + 1) * 512],
                    in_=ost,
                )
            return emit

        def outproj_units(qc):
            return [outproj_unit(qj, dn)
                    for qj in range(4 * qc, 4 * qc + 4) for dn in range(2)]

        # qkT production is sliced by token chunk n and emitted just ahead of
        # the attention block that consumes it, so ScalarE exp work starts
        # early and paces the whole kernel; spare PE slots inside attention
        # blocks are filled from a unit queue (V chunks, outproj, leftovers).
        for tn in range(4):
            qkv_v_chunk(tn)
        fq = [(lambda t: (lambda: qkv_v_chunk(t)))(tn) for tn in range(4, 8)]
        for p in range(4):
            qkv_qk_chunk(p, 0)
            qkv_qk_chunk(4 + p, 0)
            attn(0, p, fq)
        for u in fq:
            u()
        fq = [(lambda t: (lambda: qkv_v_chunk(t)))(tn) for tn in range(8, 12)]
        for p in range(4):
            qkv_qk_chunk(p, 1)
            qkv_qk_chunk(4 + p, 1)
            attn(1, p, fq)
            if p == 1:
                divisions(0)
        for u in fq:
            u()
        fq = [(lambda t: (lambda: qkv_v_chunk(t)))(tn) for tn in range(12, 16)]
        fq += outproj_units(0)
        for p in range(4):
            qkv_qk_chunk(p, 2)
            qkv_qk_chunk(4 + p, 2)
            attn(2, p, fq)
            if p == 1:
                divisions(1)
        for u in fq[:4]:
            u()
        fq = fq[4:] + outproj_units(1)
        ou2 = None
        for p in range(4):
            qkv_qk_chunk(p, 3)
            qkv_qk_chunk(4 + p, 3)
            attn(3, p, fq)
            if p == 1:
                divisions(2)
            if p == 2:
                ou2 = outproj_units(2)
                fq += ou2[:5]
        # emit the last divisions first, then drain the reserved dense units:
        # they sit ahead of the division-dependent matmuls in the PE queue and
        # keep the array busy (and HAM-warm) while the VectorE chain runs
        divisions(3)
        for u in ou2[5:]:
            u()
        for u in fq:
            u()
        for u in outproj_units(3):
            u()

    _split_multi_waits(nc, mybir)
    return nc


def _make_masks():
    kl = np.arange(128)[:, None]
    ql = np.arange(512)[None, :]
    t = [(ql >= kl + 128 * i).astype(np.float32) for i in range(4)]
    # block dg holds the mask for diagonal offset 128*dg, duplicated for the
    # two heads packed side by side in each [128, 1024] score group
    return np.concatenate([np.concatenate([m, m], axis=1) for m in t],
                          axis=1).astype(bf16)  # [128, 4096]


def _make_in_maps(x, W_qkv, b_qkv, W_out):
    masks = _make_masks()
    # oh[k, (4p+a)*128 + c] = (k == 4*(2p + (c >= 64)) + a): K=32 one-hot
    # stationary that replicates head-pair p's reciprocal rows (stored as
    # [4, 128] blocks per head) across 128 psum partitions, one query
    # segment a at a time
    oh = np.zeros((32, 4, 4, 128), np.float32)
    for p in range(4):
        for a in range(4):
            oh[4 * (2 * p) + a, p, a, 0:64] = 1.0
            oh[4 * (2 * p + 1) + a, p, a, 64:128] = 1.0
    oh = oh.reshape(32, 16 * 128).astype(bf16)
    in_maps = []
    for c in range(NCORES):
        b, g = divmod(c, 2)
        xT = np.ascontiguousarray(x[b].T).astype(bf16)
        wq = W_qkv[:, g * DG:(g + 1) * DG]
        wk = W_qkv[:, D + g * DG:D + (g + 1) * DG]
        wv = W_qkv[:, 2 * D + g * DG:2 * D + (g + 1) * DG]
        wqk = np.concatenate([wq, wk], axis=1).astype(bf16)
        bq = b_qkv[g * DG:(g + 1) * DG]
        bk = b_qkv[D + g * DG:D + (g + 1) * DG]
        bqk = np.concatenate([bq, bk]).astype(np.float32)
        wo = W_out[g * DG:(g + 1) * DG, :].astype(bf16)
        in_maps.append({
            "xT": xT,
            "wqk": wqk,
            "wv": np.ascontiguousarray(wv).astype(bf16),
            "wo": np.ascontiguousarray(wo),
            "bqk": bqk,
            "masks": masks,
            "oh": oh,
        })
    return in_maps


def _np_fallback(x, W_qkv, b_qkv, W_out, b_out):
    out = np.empty((B, T, D), np.float32)
    qkv = x.reshape(B * T, D) @ W_qkv + b_qkv
    q, k, v = np.split(qkv.reshape(B, T, 3 * D), 3, axis=-1)

    def heads(z):
        return z.reshape(B, T, H, HD).transpose(0, 2, 1, 3)

    q, k, v = heads(q), heads(k), heads(v)
    causal = np.tril(np.ones((T, T), dtype=bool))
    acc = np.empty((B, H, T, HD), np.float32)
    for bi in range(B):
        for h in range(H):
            s = (q[bi, h] @ k[bi, h].T) * np.float32(SCALE)
            s = np.where(causal, s, -np.inf)
            s -= s.max(axis=-1, keepdims=True)
            p = np.exp(s)
            p /= p.sum(axis=-1, keepdims=True)
            acc[bi, h] = p @ v[bi, h]
    a = acc.transpose(0, 2, 1, 3).reshape(B, T, D)
    for bi in range(B):
        out[bi] = a[bi] @ W_out + b_out
    return out


def run(x, W_qkv, b_qkv, W_out, b_out, trace=False, trace_kwargs=None):
    from concourse import bass_utils

    x = np.asarray(x, np.float32)
    W_qkv = np.asarray(W_qkv, np.float32)
    b_qkv = np.asarray(b_qkv, np.float32)
    W_out = np.asarray(W_out, np.float32)
    b_out = np.asarray(b_out, np.float32)

    # the on-device kernel folds b_qkv's q/k slices in; its v slice is
    # assumed zero (true for this problem family). Fall back if not.
    if np.any(b_qkv[2 * D:]):
        return _np_fallback(x, W_qkv, b_qkv, W_out, b_out), None

    if "nc" not in _CACHE:
        _CACHE["nc"] = _build_bass()
    nc = _CACHE["nc"]

    in_maps = _make_in_maps(x, W_qkv, b_qkv, W_out)
    kw = dict(trace=trace)
    if trace_kwargs:
        kw.update(trace_kwargs)
    res = bass_utils.run_bass_kernel_spmd(nc, in_maps, list(range(NCORES)), **kw)

    out = np.empty((B, T, D), np.float32)
    for b in range(B):
        out[b] = (np.asarray(res.results[2 * b]["out"], np.float32)
                  + np.asarray(res.results[2 * b + 1]["out"], np.float32)
                  + b_out)
    return out, res


def kernel(x, W_qkv, b_qkv, W_out, b_out):
    out, _ = run(x, W_qkv, b_qkv, W_out, b_out, trace=False)
    return out


# revision 20
# speedup vs baseline: 1.4166x; 1.0869x over previous
"""Causal self-attention on 8 TRN2 NeuronCores.

Sharding: core c = (batch b = c // 2, head-group g = c % 2).
Each core handles one batch and 8 of the 16 heads:
  - QKV projection for its 512 q/k/v feature slices (transposed layout)
  - causal attention for its 8 heads
  - partial output projection (its 512 rows of W_out)
Host sums the two partials per batch and adds b_out.

All TensorE matmuls run in bf16; softmax runs in f32 (exp on ScalarE,
normalization via ones-matmul column sums + VectorE reciprocal).

PE array tiling:
  - scores: the two heads of a pair are row-tiled (K=64 lhsT at partitions
    0-63 / 64-127) and run concurrently in the top/bottom array halves.
  - AV: the two heads are col-tiled (M=64 outputs at psum partitions
    0-63 / 64-127) and run concurrently in the left/right array halves.
  - softmax denominators: M=1 ones-matmuls accumulate into four distinct
    32-col groups (head x kc-parity) of one psum bank, so consecutive
    chunks' denominator matmuls also overlap.
  - normalization: denominator reciprocals for the 8 heads live on 8 SBUF
    partitions; one K=8 one-hot matmul per head pair replicates them
    across 128 psum partitions for the VectorE multiply.
"""

import numpy as np
import ml_dtypes

B, T, D, H = 4, 2048, 1024, 16
HG = 2            # head groups (tensor-parallel factor)
HL = H // HG      # 8 heads per core
HD = D // H       # 64
DG = HL * HD      # 512 features per group
SCALE = 1.0 / float(np.sqrt(HD))
NCORES = 8
TCH = T // 128    # 16 time chunks of 128
NQC = T // 512    # 4 query chunks of 512

bf16 = ml_dtypes.bfloat16

_CACHE = {}


def _split_multi_waits(nc, mybir):
    """The TPB instruction encoding has a single wait slot; this walrus build
    rejects instructions carrying more than one sync wait. Hoist extra waits
    onto standalone EventSemaphore instructions on the same engine. Tile's
    schedule is a valid serialization (waits only reference earlier-ordered
    work on other streams), so blocking the issuing stream at the same point
    cannot deadlock."""
    SKIP = ("InstTriggerDma", "InstCollectiveCompute")
    for f in nc.m.functions:
        for blk in f.blocks:
            out = []
            changed = False
            for inst in blk.instructions:
                si = getattr(inst, "sync_info", None)
                ow = list(si.on_wait) if si is not None and si.on_wait else []
                if len(ow) > 1 and type(inst).__name__ not in SKIP:
                    for i, w in enumerate(ow[:-1]):
                        out.append(mybir.InstEventSemaphore(
                            name=f"{inst.name}_hw{i}",
                            engine=inst.engine,
                            sync_info=mybir.SyncInfo(on_wait=[w], on_update=[]),
                            bass_nofuse=True,
                        ))
                    inst.sync_info = mybir.SyncInfo(
                        on_wait=[ow[-1]],
                        on_update=list(si.on_update) if si.on_update else [],
                    )
                    changed = True
                out.append(inst)
            if changed:
                blk.instructions = out


def _build_bass():
    import concourse.bass as bass
    import concourse.mybir as mybir
    import concourse.tile as tile
    from contextlib import ExitStack

    dt = mybir.dt
    f32 = dt.float32
    bf = dt.bfloat16

    nc = bass.Bass()
    xT_d = nc.declare_dram_parameter("xT", [D, T], bf, isOutput=False)
    wqk_d = nc.declare_dram_parameter("wqk", [D, 2 * DG], bf, isOutput=False)
    wv_d = nc.declare_dram_parameter("wv", [D, DG], bf, isOutput=False)
    wo_d = nc.declare_dram_parameter("wo", [DG, D], bf, isOutput=False)
    bqk_d = nc.declare_dram_parameter("bqk", [2 * DG], f32, isOutput=False)
    masks_d = nc.declare_dram_parameter("masks", [128, 4096], bf, isOutput=False)
    oh_d = nc.declare_dram_parameter("oh", [32, 16 * 128], bf, isOutput=False)
    out_d = nc.declare_dram_parameter("out", [T, D], f32, isOutput=True)

    with tile.TileContext(nc) as tc, ExitStack() as ctx:
        const = ctx.enter_context(tc.tile_pool(name="const", bufs=1))
        psum = ctx.enter_context(tc.tile_pool(name="psum", bufs=2, space="PSUM"))
        ptp = ctx.enter_context(tc.tile_pool(name="ptp", bufs=5))
        stp = ctx.enter_context(tc.tile_pool(name="stp", bufs=10))
        small = ctx.enter_context(tc.tile_pool(name="small", bufs=3))

        # ---- resident tensors --------------------------------------------
        xT_sb = const.tile([128, 8, T], bf)          # x[b].T   (feature-major)
        wqk_sb = const.tile([128, 8, 2 * DG], bf)    # W_qkv q|k columns
        wv_sb = const.tile([128, 8, DG], bf)         # W_qkv v columns
        wo_sb = const.tile([128, 4, D], bf)          # W_out rows for group
        qkT_sb = const.tile([128, 8, T], bf)         # [q^T | k^T]  (feature-major)
        vn_sb = const.tile([128, TCH, DG], bf)       # V (key-major, 8 heads x 64)
        at_sb = const.tile([128, 4, T], bf)          # A^T (normalized attn out)
        masks_sb = const.tile([128, 4096], bf)       # per-dg diagonal masks x2 heads
        bqk_sb = const.tile([128, 8], f32)
        oh_sb = const.tile([32, 16 * 128], bf)       # K=32 one-hot lhsT per (pair, seg)
        onecol = const.tile([128, 1], bf)            # all-ones stationary column

        nc.vector.memset(onecol, 1.0)

        # initial loads, striped across three DMA queues in dependency order:
        # the first V/QK matmuls wait on ~4MB, not the full 9.5MB working set
        queues = [nc.sync, nc.gpsimd, nc.scalar]
        group_a = ([(xT_sb[:, c, 0:512], xT_d[c * 128:(c + 1) * 128, 0:512])
                    for c in range(8)]
                   + [(wv_sb[:, c, :], wv_d[c * 128:(c + 1) * 128, :])
                      for c in range(8)])
        group_b = ([(wqk_sb[:, c, :], wqk_d[c * 128:(c + 1) * 128, :])
                    for c in range(8)]
                   + [(bqk_sb, bqk_d[:].rearrange("(c p) -> p c", p=128)),
                      (oh_sb, oh_d[:, :]),
                      (masks_sb, masks_d[:, :])]
                   + [(xT_sb[:, c, 512:1024], xT_d[c * 128:(c + 1) * 128, 512:1024])
                      for c in range(8)]
                   + [(xT_sb[:, c, 1024:2048], xT_d[c * 128:(c + 1) * 128, 1024:2048])
                      for c in range(8)]
                   + [(wo_sb[:, c, :], wo_d[c * 128:(c + 1) * 128, :])
                      for c in range(4)])
        for i, (dst, src) in enumerate(group_a + group_b):
            queues[i % 3].dma_start(out=dst, in_=src)

        def qkv_v_chunk(tn):
            pv = psum.tile([128, 512], f32, tag="mm512", name=f"pv{tn}")
            for k in range(8):
                nc.tensor.matmul(
                    pv,
                    lhsT=xT_sb[:, k, tn * 128:(tn + 1) * 128],
                    rhs=wv_sb[:, k, :],
                    start=(k == 0), stop=(k == 7),
                )
            nc.vector.tensor_copy(out=vn_sb[:, tn, :], in_=pv)

        def qkv_qk_chunk(m, n):
            """Produce qkT features m*128..m*128+128 for token slice n."""
            pq = psum.tile([128, 512], f32, tag="mm512", name=f"pq{m}_{n}")
            for k in range(8):
                nc.tensor.matmul(
                    pq,
                    lhsT=wqk_sb[:, k, m * 128:(m + 1) * 128],
                    rhs=xT_sb[:, k, n * 512:(n + 1) * 512],
                    start=(k == 0), stop=(k == 7),
                )
            # psum -> SBUF bf16 with per-partition bias add, on VectorE so the
            # ScalarE stays dedicated to softmax exps
            nc.vector.tensor_scalar_add(
                out=qkT_sb[:, m, n * 512:(n + 1) * 512],
                in0=pq, scalar1=bqk_sb[:, m:m + 1],
            )

        # ---- attention (interleaved with QKV production) -----------------
        stages = {}
        ge8 = {}
        go8 = {}

        def attn(qc, p, fill=None):
            """Scores + AV + denominators for head pair p of query chunk qc.

            Each score group is one kc for both heads ([128, 1024] psum, two
            concurrent row-tiled K=64 matmuls); ScalarE exps it into pt.
            AV (col-tiled pair) and denominator matmuls for groups g-2, g-1
            are emitted after group g's score matmuls so the PE stream always
            has work while exp runs."""
            nkc = 4 * qc + 4
            h0, h1 = 2 * p, 2 * p + 1
            if p == 0:
                ge8[qc] = stp.tile([32, 128], f32, tag="ge8", bufs=2,
                                   name=f"ge8_{qc}")
                go8[qc] = stp.tile([32, 128], f32, tag="go8", bufs=2,
                                   name=f"go8_{qc}")
            qsl0 = qkT_sb[0:64, p, qc * 512:(qc + 1) * 512]
            qsl1 = qkT_sb[64:128, p, qc * 512:(qc + 1) * 512]
            pts = []
            ptsums = []
            pav = psum.tile([128, 512], f32, tag="av", name=f"pav{qc}_{p}")
            pones = psum.tile([128, 512], f32, tag="av", name=f"pones{qc}_{p}")

            def av_group(kc):
                dg = kc - (nkc - 4)
                lo = 128 * dg if dg >= 1 else 0
                nc.tensor.matmul(
                    pav[0:64, lo:512],
                    lhsT=vn_sb[:, kc, h0 * HD:(h0 + 1) * HD],
                    rhs=pts[kc][:, lo:512],
                    start=(kc == 0), stop=(kc == nkc - 1),
                )
                nc.tensor.matmul(
                    pav[64:128, lo:512],
                    lhsT=vn_sb[:, kc, h1 * HD:(h1 + 1) * HD],
                    rhs=pts[kc][:, 512 + lo:1024],
                    start=(kc == 0), stop=(kc == nkc - 1),
                )

            def ones_pair(j):
                # denominator matmuls over the VectorE-precomputed pt pair
                # sum; head x pair-parity -> 4 distinct col groups (psum rows
                # 0/32/64/96) so consecutive pairs' matmuls overlap
                np_ = nkc // 2
                j0, j1 = j % 2, 2 + j % 2
                nc.tensor.matmul(
                    pones[32 * j0:32 * j0 + 1, :],
                    lhsT=onecol, rhs=ptsums[j][:, 0:512],
                    start=(j < 2), stop=(j >= np_ - 2),
                    tile_position=(0, 32 * j0),
                )
                nc.tensor.matmul(
                    pones[32 * j1:32 * j1 + 1, :],
                    lhsT=onecol, rhs=ptsums[j][:, 512:1024],
                    start=(j < 2), stop=(j >= np_ - 2),
                    tile_position=(0, 32 * j1),
                )

            def batch(kc):
                ptsum = ptp.tile([128, 1024], bf, tag="ptsum", bufs=3,
                                 name=f"pts{qc}_{p}_{kc}")
                nc.vector.tensor_add(out=ptsum, in0=pts[kc], in1=pts[kc + 1])
                ptsums.append(ptsum)
                if fill:
                    # dense fill ahead of the AV groups: it runs while the
                    # trailing chunk's exp finishes
                    fill.pop(0)()
                av_group(kc)
                av_group(kc + 1)
                if kc >= 2:
                    # trail by one batch so the PE never waits on VectorE
                    ones_pair(kc // 2 - 1)

            for kc in range(nkc):
                dg = kc - (nkc - 4)  # 0..3 on the masked diagonal band
                slo = 128 * dg if dg >= 1 else 0
                ps = psum.tile([128, 1024], f32, tag="s", name=f"ps{qc}_{p}_{kc}")
                nc.tensor.matmul(
                    ps[:, slo:512],
                    lhsT=qkT_sb[0:64, 4 + p, kc * 128:(kc + 1) * 128],
                    rhs=qsl0[:, slo:512], start=True, stop=True,
                )
                nc.tensor.matmul(
                    ps[:, 512 + slo:1024],
                    lhsT=qkT_sb[64:128, 4 + p, kc * 128:(kc + 1) * 128],
                    rhs=qsl1[:, slo:512], start=True, stop=True,
                )
                if kc > 0 and kc % 2 == 0:
                    batch(kc - 2)
                pt = ptp.tile([128, 1024], bf, tag="pt", name=f"pt{qc}_{p}_{kc}")
                ptv = pt.rearrange("p (h c) -> p h c", c=512)
                psv = ps.rearrange("p (h c) -> p h c", c=512)
                if dg >= 1:
                    # diagonal tiles: exp only the causally-live columns
                    lo = 128 * dg
                    nc.vector.memset(ptv[:, :, 0:lo], 0.0)
                    nc.scalar.activation(
                        out=ptv[:, :, lo:512], in_=psv[:, :, lo:512],
                        func=mybir.ActivationFunctionType.Exp, scale=SCALE,
                    )
                else:
                    nc.scalar.activation(
                        out=pt, in_=ps,
                        func=mybir.ActivationFunctionType.Exp, scale=SCALE,
                    )
                if dg >= 0:
                    # only 128 columns per head straddle the diagonal; the
                    # rest are fully live (or already memset to zero)
                    lo = 128 * dg
                    mv = masks_sb[:, dg * 1024:(dg + 1) * 1024].rearrange(
                        "p (h c) -> p h c", c=512)
                    nc.vector.tensor_mul(
                        out=ptv[:, :, lo:lo + 128], in0=ptv[:, :, lo:lo + 128],
                        in1=mv[:, :, lo:lo + 128],
                    )
                pts.append(pt)
            batch(nkc - 2)
            ones_pair(nkc // 2 - 1)

            stage = stp.tile([128, 512], bf, tag="stage", bufs=6,
                             name=f"st{qc}_{p}")
            nc.vector.tensor_copy(out=stage, in_=pav)
            stages[(qc, p)] = stage
            # denominator partials (4 col-group rows) -> SBUF, then gather the
            # strided rows as [4, 128] blocks so the add/reciprocal run wide
            pone_sb = stp.tile([97, 512], f32, tag="pone", bufs=2,
                               name=f"pone{qc}_{p}")
            nc.vector.tensor_copy(out=pone_sb, in_=pones[0:97, :])
            for r, buf, q in ((0, ge8, nc.gpsimd), (32, go8, nc.gpsimd),
                              (64, ge8, nc.sync), (96, go8, nc.sync)):
                h = (r // 64) & 1
                q.dma_start(
                    out=buf[qc][8 * p + 4 * h:8 * p + 4 * h + 4, :],
                    in_=pone_sb[r:r + 1, :].rearrange("o (a b) -> o a b", b=128),
                )

        def divisions(qc):
            coll = stp.tile([32, 128], f32, tag="coll", bufs=2, name=f"coll{qc}")
            nc.vector.tensor_add(out=coll, in0=ge8[qc], in1=go8[qc])
            rcoll = stp.tile([32, 128], f32, tag="rcoll", bufs=2)
            nc.vector.reciprocal(rcoll, coll)
            rcollb = stp.tile([32, 128], bf, tag="rcollb", bufs=2)
            nc.vector.tensor_copy(out=rcollb, in_=rcoll)
            for p in range(4):
                # replicate rows 8p..8p+8 of rcollb across 128 psum
                # partitions via K=32 one-hot stationary matmuls (one per
                # 128-query segment)
                prb = psum.tile([128, 512], f32, tag="mm512", name=f"prb{qc}_{p}")
                for a in range(4):
                    nc.tensor.matmul(
                        prb[:, a * 128:(a + 1) * 128],
                        lhsT=oh_sb[:, (p * 4 + a) * 128:(p * 4 + a + 1) * 128],
                        rhs=rcollb, start=True, stop=True,
                    )
                nc.vector.tensor_mul(
                    out=at_sb[:, p, qc * 512:(qc + 1) * 512],
                    in0=stages[(qc, p)], in1=prb,
                )

        def outproj_unit(qj, dn):
            def emit():
                po = psum.tile([128, 512], f32, tag="mm512",
                               name=f"po{qj}_{dn}")
                for kc in range(4):
                    nc.tensor.matmul(
                        po,
                        lhsT=at_sb[:, kc, qj * 128:(qj + 1) * 128],
                        rhs=wo_sb[:, kc, dn * 512:(dn + 1) * 512],
                        start=(kc == 0), stop=(kc == 3),
                    )
                ost = small.tile([128, 512], f32, tag="ost")
                nc.vector.tensor_copy(out=ost, in_=po)
                nc.sync.dma_start(
                    out=out_d[qj * 128:(qj + 1) * 128,
                              dn * 512:(dn + 1) * 512],
                    in_=ost,
                )
            return emit

        def outproj_units(qc):
            return [outproj_unit(qj, dn)
                    for qj in range(4 * qc, 4 * qc + 4) for dn in range(2)]

        # qkT production, V chunks, and outproj all ride the attention fill
        # queue: one dense unit per 2-chunk batch, emitted so that pair p's
        # qkT units are always consumed during pair p-1 (the in-order PE
        # queue would deadlock otherwise). This keeps ScalarE exp streaming
        # across pair boundaries instead of idling behind dense QKV blocks.
        def v_unit(tn):
            return lambda: qkv_v_chunk(tn)

        def qk_units(qc):
            us = []
            for p in range(1, 4):
                us += [lambda m=p, n=qc: qkv_qk_chunk(m, n),
                       lambda m=4 + p, n=qc: qkv_qk_chunk(m, n)]
            if qc < 3:
                us += [lambda n=qc + 1: qkv_qk_chunk(0, n),
                       lambda n=qc + 1: qkv_qk_chunk(4, n)]
            return us

        for tn in range(4):
            qkv_v_chunk(tn)
        qkv_qk_chunk(0, 0)
        qkv_qk_chunk(4, 0)
        fq = qk_units(0)
        for p in range(4):
            attn(0, p, fq)
        for u in fq:
            u()
        for tn in range(4, 8):
            qkv_v_chunk(tn)
        fq = qk_units(1) + [v_unit(tn) for tn in range(8, 12)]
        for p in range(4):
            attn(1, p, fq)
            if p == 1:
                divisions(0)
        for u in fq:
            u()
        fq = qk_units(2) + [v_unit(tn) for tn in range(12, 16)]
        fq += outproj_units(0)
        for p in range(4):
            attn(2, p, fq)
            if p == 1:
                divisions(1)
        for u in fq:
            u()
        fq = qk_units(3) + outproj_units(1)
        ou2 = None
        for p in range(4):
            attn(3, p, fq)
            if p == 1:
                divisions(2)
            if p == 2:
                ou2 = outproj_units(2)
                fq += ou2[:5]
        # emit the last divisions first, then drain the reserved dense units:
        # they sit ahead of the division-dependent matmuls in the PE queue and
        # keep the array busy (and HAM-warm) while the VectorE chain runs
        divisions(3)
        for u in ou2[5:]:
            u()
        for u in fq:
            u()
        for u in outproj_units(3):
            u()

    _split_multi_waits(nc, mybir)
    return nc


def _make_masks():
    kl = np.arange(128)[:, None]
    ql = np.arange(512)[None, :]
    t = [(ql >= kl + 128 * i).astype(np.float32) for i in range(4)]
    # block dg holds the mask for diagonal offset 128*dg, duplicated for the
    # two heads packed side by side in each [128, 1024] score group
    return np.concatenate([np.concatenate([m, m], axis=1) for m in t],
                          axis=1).astype(bf16)  # [128, 4096]


def _make_in_maps(x, W_qkv, b_qkv, W_out):
    masks = _make_masks()
    # oh[k, (4p+a)*128 + c] = (k == 4*(2p + (c >= 64)) + a): K=32 one-hot
    # stationary that replicates head-pair p's reciprocal rows (stored as
    # [4, 128] blocks per head) across 128 psum partitions, one query
    # segment a at a time
    oh = np.zeros((32, 4, 4, 128), np.float32)
    for p in range(4):
        for a in range(4):
            oh[4 * (2 * p) + a, p, a, 0:64] = 1.0
            oh[4 * (2 * p + 1) + a, p, a, 64:128] = 1.0
    oh = oh.reshape(32, 16 * 128).astype(bf16)
    in_maps = []
    for c in range(NCORES):
        b, g = divmod(c, 2)
        xT = np.ascontiguousarray(x[b].T).astype(bf16)
        wq = W_qkv[:, g * DG:(g + 1) * DG]
        wk = W_qkv[:, D + g * DG:D + (g + 1) * DG]
        wv = W_qkv[:, 2 * D + g * DG:2 * D + (g + 1) * DG]
        wqk = np.concatenate([wq, wk], axis=1).astype(bf16)
        bq = b_qkv[g * DG:(g + 1) * DG]
        bk = b_qkv[D + g * DG:D + (g + 1) * DG]
        bqk = np.concatenate([bq, bk]).astype(np.float32)
        wo = W_out[g * DG:(g + 1) * DG, :].astype(bf16)
        in_maps.append({
            "xT": xT,
            "wqk": wqk,
            "wv": np.ascontiguousarray(wv).astype(bf16),
            "wo": np.ascontiguousarray(wo),
            "bqk": bqk,
            "masks": masks,
            "oh": oh,
        })
    return in_maps


def _np_fallback(x, W_qkv, b_qkv, W_out, b_out):
    out = np.empty((B, T, D), np.float32)
    qkv = x.reshape(B * T, D) @ W_qkv + b_qkv
    q, k, v = np.split(qkv.reshape(B, T, 3 * D), 3, axis=-1)

    def heads(z):
        return z.reshape(B, T, H, HD).transpose(0, 2, 1, 3)

    q, k, v = heads(q), heads(k), heads(v)
    causal = np.tril(np.ones((T, T), dtype=bool))
    acc = np.empty((B, H, T, HD), np.float32)
    for bi in range(B):
        for h in range(H):
            s = (q[bi, h] @ k[bi, h].T) * np.float32(SCALE)
            s = np.where(causal, s, -np.inf)
            s -= s.max(axis=-1, keepdims=True)
            p = np.exp(s)
            p /= p.sum(axis=-1, keepdims=True)
            acc[bi, h] = p @ v[bi, h]
    a = acc.transpose(0, 2, 1, 3).reshape(B, T, D)
    for bi in range(B):
        out[bi] = a[bi] @ W_out + b_out
    return out


def run(x, W_qkv, b_qkv, W_out, b_out, trace=False, trace_kwargs=None):
    from concourse import bass_utils

    x = np.asarray(x, np.float32)
    W_qkv = np.asarray(W_qkv, np.float32)
    b_qkv = np.asarray(b_qkv, np.float32)
    W_out = np.asarray(W_out, np.float32)
    b_out = np.asarray(b_out, np.float32)

    # the on-device kernel folds b_qkv's q/k slices in; its v slice is
    # assumed zero (true for this problem family). Fall back if not.
    if np.any(b_qkv[2 * D:]):
        return _np_fallback(x, W_qkv, b_qkv, W_out, b_out), None

    if "nc" not in _CACHE:
        _CACHE["nc"] = _build_bass()
    nc = _CACHE["nc"]

    in_maps = _make_in_maps(x, W_qkv, b_qkv, W_out)
    kw = dict(trace=trace)
    if trace_kwargs:
        kw.update(trace_kwargs)
    res = bass_utils.run_bass_kernel_spmd(nc, in_maps, list(range(NCORES)), **kw)

    out = np.empty((B, T, D), np.float32)
    for b in range(B):
        out[b] = (np.asarray(res.results[2 * b]["out"], np.float32)
                  + np.asarray(res.results[2 * b + 1]["out"], np.float32)
                  + b_out)
    return out, res


def kernel(x, W_qkv, b_qkv, W_out, b_out):
    out, _ = run(x, W_qkv, b_qkv, W_out, b_out, trace=False)
    return out


# revision 25
# speedup vs baseline: 1.4303x; 1.0097x over previous
"""Causal self-attention on 8 TRN2 NeuronCores.

Sharding: core c = (batch b = c // 2, head-group g = c % 2).
Each core handles one batch and 8 of the 16 heads:
  - QKV projection for its 512 q/k/v feature slices (transposed layout)
  - causal attention for its 8 heads
  - partial output projection (its 512 rows of W_out)
Host sums the two partials per batch and adds b_out.

All TensorE matmuls run in bf16; softmax runs in f32 (exp on ScalarE,
normalization via ones-matmul column sums + VectorE reciprocal).

PE array tiling:
  - scores: the two heads of a pair are row-tiled (K=64 lhsT at partitions
    0-63 / 64-127) and run concurrently in the top/bottom array halves.
  - AV: the two heads are col-tiled (M=64 outputs at psum partitions
    0-63 / 64-127) and run concurrently in the left/right array halves.
  - softmax denominators: M=1 ones-matmuls accumulate into four distinct
    32-col groups (head x kc-parity) of one psum bank, so consecutive
    chunks' denominator matmuls also overlap.
  - normalization: denominator reciprocals for the 8 heads live on 8 SBUF
    partitions; one K=8 one-hot matmul per head pair replicates them
    across 128 psum partitions for the VectorE multiply.
"""

import numpy as np
import ml_dtypes

B, T, D, H = 4, 2048, 1024, 16
HG = 2            # head groups (tensor-parallel factor)
HL = H // HG      # 8 heads per core
HD = D // H       # 64
DG = HL * HD      # 512 features per group
SCALE = 1.0 / float(np.sqrt(HD))
NCORES = 8
TCH = T // 128    # 16 time chunks of 128
NQC = T // 512    # 4 query chunks of 512

bf16 = ml_dtypes.bfloat16

_CACHE = {}


def _split_multi_waits(nc, mybir):
    """The TPB instruction encoding has a single wait slot; this walrus build
    rejects instructions carrying more than one sync wait. Hoist extra waits
    onto standalone EventSemaphore instructions on the same engine. Tile's
    schedule is a valid serialization (waits only reference earlier-ordered
    work on other streams), so blocking the issuing stream at the same point
    cannot deadlock."""
    SKIP = ("InstTriggerDma", "InstCollectiveCompute")
    for f in nc.m.functions:
        for blk in f.blocks:
            out = []
            changed = False
            for inst in blk.instructions:
                si = getattr(inst, "sync_info", None)
                ow = list(si.on_wait) if si is not None and si.on_wait else []
                if len(ow) > 1 and type(inst).__name__ not in SKIP:
                    for i, w in enumerate(ow[:-1]):
                        out.append(mybir.InstEventSemaphore(
                            name=f"{inst.name}_hw{i}",
                            engine=inst.engine,
                            sync_info=mybir.SyncInfo(on_wait=[w], on_update=[]),
                            bass_nofuse=True,
                        ))
                    inst.sync_info = mybir.SyncInfo(
                        on_wait=[ow[-1]],
                        on_update=list(si.on_update) if si.on_update else [],
                    )
                    changed = True
                out.append(inst)
            if changed:
                blk.instructions = out


def _build_bass():
    import concourse.bass as bass
    import concourse.mybir as mybir
    import concourse.tile as tile
    from contextlib import ExitStack

    dt = mybir.dt
    f32 = dt.float32
    bf = dt.bfloat16

    nc = bass.Bass()
    xT_d = nc.declare_dram_parameter("xT", [D, T], bf, isOutput=False)
    wqk_d = nc.declare_dram_parameter("wqk", [D, 2 * DG], bf, isOutput=False)
    wv_d = nc.declare_dram_parameter("wv", [D, DG], bf, isOutput=False)
    wo_d = nc.declare_dram_parameter("wo", [DG, D], bf, isOutput=False)
    bqk_d = nc.declare_dram_parameter("bqk", [2 * DG], f32, isOutput=False)
    masks_d = nc.declare_dram_parameter("masks", [128, 4096], bf, isOutput=False)
    oh_d = nc.declare_dram_parameter("oh", [32, 16 * 128], bf, isOutput=False)
    out_d = nc.declare_dram_parameter("out", [T, D], f32, isOutput=True)

    with tile.TileContext(nc) as tc, ExitStack() as ctx:
        const = ctx.enter_context(tc.tile_pool(name="const", bufs=1))
        psum = ctx.enter_context(tc.tile_pool(name="psum", bufs=2, space="PSUM"))
        ptp = ctx.enter_context(tc.tile_pool(name="ptp", bufs=5))
        stp = ctx.enter_context(tc.tile_pool(name="stp", bufs=10))
        small = ctx.enter_context(tc.tile_pool(name="small", bufs=3))

        # ---- resident tensors --------------------------------------------
        xT_sb = const.tile([128, 8, T], bf)          # x[b].T   (feature-major)
        wqk_sb = const.tile([128, 8, 2 * DG], bf)    # W_qkv q|k columns
        wv_sb = const.tile([128, 8, DG], bf)         # W_qkv v columns
        wo_sb = const.tile([128, 4, D], bf)          # W_out rows for group
        qkT_sb = const.tile([128, 8, T], bf)         # [q^T | k^T]  (feature-major)
        vn_sb = const.tile([128, TCH, DG], bf)       # V (key-major, 8 heads x 64)
        at_sb = const.tile([128, 4, T], bf)          # A^T (normalized attn out)
        masks_sb = const.tile([128, 4096], bf)       # per-dg diagonal masks x2 heads
        bqk_sb = const.tile([128, 8], f32)
        oh_sb = const.tile([32, 16 * 128], bf)       # K=32 one-hot lhsT per (pair, seg)
        onecol = const.tile([128, 1], bf)            # all-ones stationary column

        nc.vector.memset(onecol, 1.0)

        # initial loads, striped across three DMA queues in dependency order:
        # the first V/QK matmuls wait on ~4MB, not the full 9.5MB working set
        queues = [nc.sync, nc.gpsimd, nc.scalar]
        group_a = ([(xT_sb[:, c, 0:512], xT_d[c * 128:(c + 1) * 128, 0:512])
                    for c in range(8)]
                   + [(wv_sb[:, c, :], wv_d[c * 128:(c + 1) * 128, :])
                      for c in range(8)])
        group_b = ([(wqk_sb[:, c, :], wqk_d[c * 128:(c + 1) * 128, :])
                    for c in range(8)]
                   + [(bqk_sb, bqk_d[:].rearrange("(c p) -> p c", p=128)),
                      (oh_sb, oh_d[:, :]),
                      (masks_sb, masks_d[:, :])]
                   + [(xT_sb[:, c, 512:1024], xT_d[c * 128:(c + 1) * 128, 512:1024])
                      for c in range(8)]
                   + [(xT_sb[:, c, 1024:2048], xT_d[c * 128:(c + 1) * 128, 1024:2048])
                      for c in range(8)]
                   + [(wo_sb[:, c, :], wo_d[c * 128:(c + 1) * 128, :])
                      for c in range(4)])
        for i, (dst, src) in enumerate(group_a + group_b):
            queues[i % 3].dma_start(out=dst, in_=src)

        def qkv_v_chunk(tn):
            pv = psum.tile([128, 512], f32, tag="mm512", name=f"pv{tn}")
            for k in range(8):
                nc.tensor.matmul(
                    pv,
                    lhsT=xT_sb[:, k, tn * 128:(tn + 1) * 128],
                    rhs=wv_sb[:, k, :],
                    start=(k == 0), stop=(k == 7),
                )
            nc.vector.tensor_copy(out=vn_sb[:, tn, :], in_=pv)

        def qkv_qk_chunk(m, n):
            """Produce qkT features m*128..m*128+128 for token slice n."""
            pq = psum.tile([128, 512], f32, tag="mm512", name=f"pq{m}_{n}")
            for k in range(8):
                nc.tensor.matmul(
                    pq,
                    lhsT=wqk_sb[:, k, m * 128:(m + 1) * 128],
                    rhs=xT_sb[:, k, n * 512:(n + 1) * 512],
                    start=(k == 0), stop=(k == 7),
                )
            # psum -> SBUF bf16 with per-partition bias add, on VectorE so the
            # ScalarE stays dedicated to softmax exps
            nc.vector.tensor_scalar_add(
                out=qkT_sb[:, m, n * 512:(n + 1) * 512],
                in0=pq, scalar1=bqk_sb[:, m:m + 1],
            )

        # ---- attention (interleaved with QKV production) -----------------
        stages = {}
        ge8 = {}
        go8 = {}

        def attn(qc, p, fill=None, pending=None):
            """Scores + AV + denominators for head pair p of query chunk qc.

            Each score group is one kc for both heads ([128, 1024] psum, two
            concurrent row-tiled K=64 matmuls); ScalarE exps it into pt.
            AV (col-tiled pair) and denominator matmuls for groups g-2, g-1
            are emitted after group g's score matmuls so the PE stream always
            has work while exp runs."""
            nkc = 4 * qc + 4
            h0, h1 = 2 * p, 2 * p + 1
            if p == 0:
                ge8[qc] = stp.tile([32, 128], f32, tag="ge8", bufs=2,
                                   name=f"ge8_{qc}")
                go8[qc] = stp.tile([32, 128], f32, tag="go8", bufs=2,
                                   name=f"go8_{qc}")
            qsl0 = qkT_sb[0:64, p, qc * 512:(qc + 1) * 512]
            qsl1 = qkT_sb[64:128, p, qc * 512:(qc + 1) * 512]
            pts = []
            ptsums = []
            pav = psum.tile([128, 512], f32, tag="av", name=f"pav{qc}_{p}")
            pones = psum.tile([128, 512], f32, tag="av", name=f"pones{qc}_{p}")

            def av_group(kc):
                dg = kc - (nkc - 4)
                lo = 128 * dg if dg >= 1 else 0
                nc.tensor.matmul(
                    pav[0:64, lo:512],
                    lhsT=vn_sb[:, kc, h0 * HD:(h0 + 1) * HD],
                    rhs=pts[kc][:, lo:512],
                    start=(kc == 0), stop=(kc == nkc - 1),
                )
                nc.tensor.matmul(
                    pav[64:128, lo:512],
                    lhsT=vn_sb[:, kc, h1 * HD:(h1 + 1) * HD],
                    rhs=pts[kc][:, 512 + lo:1024],
                    start=(kc == 0), stop=(kc == nkc - 1),
                )

            def ones_pair(j):
                # denominator matmuls over the VectorE-precomputed pt pair
                # sum; head x pair-parity -> 4 distinct col groups (psum rows
                # 0/32/64/96) so consecutive pairs' matmuls overlap
                np_ = nkc // 2
                j0, j1 = j % 2, 2 + j % 2
                nc.tensor.matmul(
                    pones[32 * j0:32 * j0 + 1, :],
                    lhsT=onecol, rhs=ptsums[j][:, 0:512],
                    start=(j < 2), stop=(j >= np_ - 2),
                    tile_position=(0, 32 * j0),
                )
                nc.tensor.matmul(
                    pones[32 * j1:32 * j1 + 1, :],
                    lhsT=onecol, rhs=ptsums[j][:, 512:1024],
                    start=(j < 2), stop=(j >= np_ - 2),
                    tile_position=(0, 32 * j1),
                )

            def batch(kc):
                ptsum = ptp.tile([128, 1024], bf, tag="ptsum", bufs=3,
                                 name=f"pts{qc}_{p}_{kc}")
                nc.vector.tensor_add(out=ptsum, in0=pts[kc], in1=pts[kc + 1])
                ptsums.append(ptsum)
                if fill:
                    # dense fill ahead of the AV groups: it runs while the
                    # trailing chunk's exp finishes
                    fill.pop(0)()
                av_group(kc)
                av_group(kc + 1)
                if kc >= 2:
                    # trail by one batch so the PE never waits on VectorE
                    ones_pair(kc // 2 - 1)

            for kc in range(nkc):
                dg = kc - (nkc - 4)  # 0..3 on the masked diagonal band
                slo = 128 * dg if dg >= 1 else 0
                ps = psum.tile([128, 1024], f32, tag="s", name=f"ps{qc}_{p}_{kc}")
                nc.tensor.matmul(
                    ps[:, slo:512],
                    lhsT=qkT_sb[0:64, 4 + p, kc * 128:(kc + 1) * 128],
                    rhs=qsl0[:, slo:512], start=True, stop=True,
                )
                nc.tensor.matmul(
                    ps[:, 512 + slo:1024],
                    lhsT=qkT_sb[64:128, 4 + p, kc * 128:(kc + 1) * 128],
                    rhs=qsl1[:, slo:512], start=True, stop=True,
                )
                if kc == 2 and pending is not None:
                    # the previous pair's closing work runs here, behind our
                    # first scores, so its VectorE dependencies never stall
                    # the PE queue at the pair boundary
                    pending()
                if kc > 0 and kc % 2 == 0:
                    batch(kc - 2)
                pt = ptp.tile([128, 1024], bf, tag="pt", name=f"pt{qc}_{p}_{kc}")
                ptv = pt.rearrange("p (h c) -> p h c", c=512)
                psv = ps.rearrange("p (h c) -> p h c", c=512)
                if dg >= 1:
                    # diagonal tiles: exp only the causally-live columns
                    lo = 128 * dg
                    nc.vector.memset(ptv[:, :, 0:lo], 0.0)
                    nc.scalar.activation(
                        out=ptv[:, :, lo:512], in_=psv[:, :, lo:512],
                        func=mybir.ActivationFunctionType.Exp, scale=SCALE,
                    )
                else:
                    nc.scalar.activation(
                        out=pt, in_=ps,
                        func=mybir.ActivationFunctionType.Exp, scale=SCALE,
                    )
                if dg >= 0:
                    # only 128 columns per head straddle the diagonal; the
                    # rest are fully live (or already memset to zero)
                    lo = 128 * dg
                    mv = masks_sb[:, dg * 1024:(dg + 1) * 1024].rearrange(
                        "p (h c) -> p h c", c=512)
                    nc.vector.tensor_mul(
                        out=ptv[:, :, lo:lo + 128], in0=ptv[:, :, lo:lo + 128],
                        in1=mv[:, :, lo:lo + 128],
                    )
                pts.append(pt)
            batch(nkc - 2)

            def closing():
                ones_pair(nkc // 2 - 1)
                stage = stp.tile([128, 512], bf, tag="stage", bufs=6,
                                 name=f"st{qc}_{p}")
                nc.vector.tensor_copy(out=stage, in_=pav)
                stages[(qc, p)] = stage
                # denominator partials (4 col-group rows) -> SBUF, then gather
                # the strided rows as [4, 128] blocks so the add/reciprocal
                # run wide
                pone_sb = stp.tile([97, 512], f32, tag="pone", bufs=2,
                                   name=f"pone{qc}_{p}")
                nc.vector.tensor_copy(out=pone_sb, in_=pones[0:97, :])
                for r, buf, q in ((0, ge8, nc.gpsimd), (32, go8, nc.gpsimd),
                                  (64, ge8, nc.sync), (96, go8, nc.sync)):
                    h = (r // 64) & 1
                    q.dma_start(
                        out=buf[qc][8 * p + 4 * h:8 * p + 4 * h + 4, :],
                        in_=pone_sb[r:r + 1, :].rearrange(
                            "o (a b) -> o a b", b=128),
                    )
            return closing

        def divisions(qc):
            coll = stp.tile([32, 128], f32, tag="coll", bufs=2, name=f"coll{qc}")
            nc.vector.tensor_add(out=coll, in0=ge8[qc], in1=go8[qc])
            rcoll = stp.tile([32, 128], f32, tag="rcoll", bufs=2)
            nc.vector.reciprocal(rcoll, coll)
            rcollb = stp.tile([32, 128], bf, tag="rcollb", bufs=2)
            nc.vector.tensor_copy(out=rcollb, in_=rcoll)
            for p in range(4):
                # replicate rows 8p..8p+8 of rcollb across 128 psum
                # partitions via K=32 one-hot stationary matmuls (one per
                # 128-query segment)
                prb = psum.tile([128, 512], f32, tag="mm512", name=f"prb{qc}_{p}")
                for a in range(4):
                    nc.tensor.matmul(
                        prb[:, a * 128:(a + 1) * 128],
                        lhsT=oh_sb[:, (p * 4 + a) * 128:(p * 4 + a + 1) * 128],
                        rhs=rcollb, start=True, stop=True,
                    )
                nc.vector.tensor_mul(
                    out=at_sb[:, p, qc * 512:(qc + 1) * 512],
                    in0=stages[(qc, p)], in1=prb,
                )

        def outproj_unit(qj, dn):
            def emit():
                po = psum.tile([128, 512], f32, tag="mm512",
                               name=f"po{qj}_{dn}")
                for kc in range(4):
                    nc.tensor.matmul(
                        po,
                        lhsT=at_sb[:, kc, qj * 128:(qj + 1) * 128],
                        rhs=wo_sb[:, kc, dn * 512:(dn + 1) * 512],
                        start=(kc == 0), stop=(kc == 3),
                    )
                ost = small.tile([128, 512], f32, tag="ost")
                nc.vector.tensor_copy(out=ost, in_=po)
                nc.sync.dma_start(
                    out=out_d[qj * 128:(qj + 1) * 128,
                              dn * 512:(dn + 1) * 512],
                    in_=ost,
                )
            return emit

        def outproj_units(qc):
            return [outproj_unit(qj, dn)
                    for qj in range(4 * qc, 4 * qc + 4) for dn in range(2)]

        # qkT production, V chunks, and outproj all ride the attention fill
        # queue: one dense unit per 2-chunk batch, emitted so that pair p's
        # qkT units are always consumed during pair p-1 (the in-order PE
        # queue would deadlock otherwise). This keeps ScalarE exp streaming
        # across pair boundaries instead of idling behind dense QKV blocks.
        def v_unit(tn):
            return lambda: qkv_v_chunk(tn)

        def qk_units(qc):
            us = []
            for p in range(1, 4):
                us += [lambda m=p, n=qc: qkv_qk_chunk(m, n),
                       lambda m=4 + p, n=qc: qkv_qk_chunk(m, n)]
            if qc < 3:
                us += [lambda n=qc + 1: qkv_qk_chunk(0, n),
                       lambda n=qc + 1: qkv_qk_chunk(4, n)]
            return us

        # ~4us of junk matmuls during the initial DMA wait bring the PE HAM
        # clock gate to full rate before the first real chunk arrives
        warm_sb = const.tile([128, 384], bf)
        nc.vector.memset(warm_sb, 0.0)
        ps_warm = psum.tile([128, 1024], f32, tag="s", name="ps_warm")
        for _ in range(48):
            nc.tensor.matmul(ps_warm[:, 0:256], lhsT=warm_sb[:, 0:128],
                             rhs=warm_sb[:, 128:384], start=True, stop=True)

        pend = None
        for tn in range(4):
            qkv_v_chunk(tn)
        qkv_qk_chunk(0, 0)
        qkv_qk_chunk(4, 0)
        fq = qk_units(0)
        for p in range(4):
            pend = attn(0, p, fq, pend)
        for u in fq:
            u()
        for tn in range(4, 8):
            qkv_v_chunk(tn)
        fq = qk_units(1) + [v_unit(tn) for tn in range(8, 12)]
        for p in range(4):
            pend = attn(1, p, fq, pend)
            if p == 1:
                divisions(0)
        for u in fq:
            u()
        fq = qk_units(2) + [v_unit(tn) for tn in range(12, 16)]
        fq += outproj_units(0)
        for p in range(4):
            pend = attn(2, p, fq, pend)
            if p == 1:
                divisions(1)
        for u in fq:
            u()
        fq = qk_units(3) + outproj_units(1)
        ou2 = None
        for p in range(4):
            pend = attn(3, p, fq, pend)
            if p == 1:
                divisions(2)
            if p == 2:
                ou2 = outproj_units(2)
                fq += ou2[:2]
        pend()
        # reserved dense units go ahead of the division-dependent matmuls in
        # the PE queue: they keep the array busy (and HAM-warm) while the
        # last divisions' VectorE chain runs
        for u in ou2[2:]:
            u()
        for u in fq:
            u()
        divisions(3)
        for u in outproj_units(3):
            u()

    _split_multi_waits(nc, mybir)
    return nc


def _make_masks():
    kl = np.arange(128)[:, None]
    ql = np.arange(512)[None, :]
    t = [(ql >= kl + 128 * i).astype(np.float32) for i in range(4)]
    # block dg holds the mask for diagonal offset 128*dg, duplicated for the
    # two heads packed side by side in each [128, 1024] score group
    return np.concatenate([np.concatenate([m, m], axis=1) for m in t],
                          axis=1).astype(bf16)  # [128, 4096]


def _make_in_maps(x, W_qkv, b_qkv, W_out):
    masks = _make_masks()
    # oh[k, (4p+a)*128 + c] = (k == 4*(2p + (c >= 64)) + a): K=32 one-hot
    # stationary that replicates head-pair p's reciprocal rows (stored as
    # [4, 128] blocks per head) across 128 psum partitions, one query
    # segment a at a time
    oh = np.zeros((32, 4, 4, 128), np.float32)
    for p in range(4):
        for a in range(4):
            oh[4 * (2 * p) + a, p, a, 0:64] = 1.0
            oh[4 * (2 * p + 1) + a, p, a, 64:128] = 1.0
    oh = oh.reshape(32, 16 * 128).astype(bf16)
    in_maps = []
    for c in range(NCORES):
        b, g = divmod(c, 2)
        xT = np.ascontiguousarray(x[b].T).astype(bf16)
        wq = W_qkv[:, g * DG:(g + 1) * DG]
        wk = W_qkv[:, D + g * DG:D + (g + 1) * DG]
        wv = W_qkv[:, 2 * D + g * DG:2 * D + (g + 1) * DG]
        wqk = np.concatenate([wq, wk], axis=1).astype(bf16)
        bq = b_qkv[g * DG:(g + 1) * DG]
        bk = b_qkv[D + g * DG:D + (g + 1) * DG]
        bqk = np.concatenate([bq, bk]).astype(np.float32)
        wo = W_out[g * DG:(g + 1) * DG, :].astype(bf16)
        in_maps.append({
            "xT": xT,
            "wqk": wqk,
            "wv": np.ascontiguousarray(wv).astype(bf16),
            "wo": np.ascontiguousarray(wo),
            "bqk": bqk,
            "masks": masks,
            "oh": oh,
        })
    return in_maps


def _np_fallback(x, W_qkv, b_qkv, W_out, b_out):
    out = np.empty((B, T, D), np.float32)
    qkv = x.reshape(B * T, D) @ W_qkv + b_qkv
    q, k, v = np.split(qkv.reshape(B, T, 3 * D), 3, axis=-1)

    def heads(z):
        return z.reshape(B, T, H, HD).transpose(0, 2, 1, 3)

    q, k, v = heads(q), heads(k), heads(v)
    causal = np.tril(np.ones((T, T), dtype=bool))
    acc = np.empty((B, H, T, HD), np.float32)
    for bi in range(B):
        for h in range(H):
            s = (q[bi, h] @ k[bi, h].T) * np.float32(SCALE)
            s = np.where(causal, s, -np.inf)
            s -= s.max(axis=-1, keepdims=True)
            p = np.exp(s)
            p /= p.sum(axis=-1, keepdims=True)
            acc[bi, h] = p @ v[bi, h]
    a = acc.transpose(0, 2, 1, 3).reshape(B, T, D)
    for bi in range(B):
        out[bi] = a[bi] @ W_out + b_out
    return out


def run(x, W_qkv, b_qkv, W_out, b_out, trace=False, trace_kwargs=None):
    from concourse import bass_utils

    x = np.asarray(x, np.float32)
    W_qkv = np.asarray(W_qkv, np.float32)
    b_qkv = np.asarray(b_qkv, np.float32)
    W_out = np.asarray(W_out, np.float32)
    b_out = np.asarray(b_out, np.float32)

    # the on-device kernel folds b_qkv's q/k slices in; its v slice is
    # assumed zero (true for this problem family). Fall back if not.
    if np.any(b_qkv[2 * D:]):
        return _np_fallback(x, W_qkv, b_qkv, W_out, b_out), None

    if "nc" not in _CACHE:
        _CACHE["nc"] = _build_bass()
    nc = _CACHE["nc"]

    in_maps = _make_in_maps(x, W_qkv, b_qkv, W_out)
    kw = dict(trace=trace)
    if trace_kwargs:
        kw.update(trace_kwargs)
    res = bass_utils.run_bass_kernel_spmd(nc, in_maps, list(range(NCORES)), **kw)

    out = np.empty((B, T, D), np.float32)
    for b in range(B):
        out[b] = (np.asarray(res.results[2 * b]["out"], np.float32)
                  + np.asarray(res.results[2 * b + 1]["out"], np.float32)
                  + b_out)
    return out, res


def kernel(x, W_qkv, b_qkv, W_out, b_out):
    out, _ = run(x, W_qkv, b_qkv, W_out, b_out, trace=False)
    return out
